# revision 1
# baseline (speedup 1.0000x reference)
"""MultiHeadGraphAttention kernel for 8 Trainium2 NeuronCores.

Sharding (2D): 4 src-quarters x 2 dst-halves. Device (q, half) owns edges
with src in quarter q (12544 nodes = 98 blocks of 128) and dst in half
(25024 rows). Each device gathers x rows from its 25024-row half-table via
the GPSIMD dma_gather custom op (int16 indices fit the half-table), and
produces PARTIAL per-head aggregates for its node quarter. Host sums the
half pairs, divides by host-computed rowsums and applies the per-head diag
weight w.

Per 128-edge tile (edges sorted by src within a 128-node block):
  W_h[j,i] = (seg_rel[j] == i) * ee[h,j]      (one fused DVE tensor_scalar
                                               per head: is_equal then mult)
  PSUM[f,(h,i)] += Xg.T @ [W_1|W_2|W_3|W_4]   (PE matmul, bf16, Xg stationary)
PSUM accumulates over a block's tiles; the transposed block aggregate is
DMA'd out raw and host fixes the layout.

Edge scores ssum[e,h] = s_src[src_e,h] + s_dst[dst_e,h] are precomputed on
host (s = x @ (w*a) is a tiny [N,4] projection); the device computes
ee = exp(-leaky_relu(ssum)) in batched DVE/ACT ops.
"""

import sys

sys.path.insert(0, "/opt/trn_rl_repo")

import ml_dtypes
import numpy as np

import concourse.bass as bass
import concourse.tile as tile
from concourse import bacc, mybir
from concourse.bass_utils import run_bass_kernel_spmd
from concourse.library_config import mlp

N_NODES = 50000
H = 4
F = 128
P = 128
NCORES = 8
NQ = 4                      # src quarters
B_PER_DEV = 98              # node blocks per quarter (98*128 = 12544)
NODES_Q = B_PER_DEV * P     # 12544
HALF = 25024                # dst half-table rows (2*25024 = 50048 >= 50000)
PAD_SCORE = 40.0            # exp(-40) ~ 4e-18: padding edges add nothing

_last_results = None  # test.py introspection (exec_time_ns etc.)
_program_cache = {}


def _build_program(t_pb: int):
    """SPMD program, identical on all 8 cores; t_pb = edge tiles per block."""
    f32 = mybir.dt.float32
    bf16 = mybir.dt.bfloat16
    i16 = mybir.dt.int16
    T = B_PER_DEV * t_pb

    nc = bacc.Bacc("TRN2", target_bir_lowering=False, debug=False,
                   num_devices=NCORES)

    xtab = nc.dram_tensor("xtab", [HALF, F], bf16, kind="ExternalInput").ap()
    idxw = nc.dram_tensor("idxw", [P, T * 8], i16, kind="ExternalInput").ap()
    segt = nc.dram_tensor("segt", [P, T], f32, kind="ExternalInput").ap()
    ssum = nc.dram_tensor("ssum", [P, 4 * T], f32, kind="ExternalInput").ap()
    iota = nc.dram_tensor("iota", [P, P], bf16, kind="ExternalInput").ap()
    aggt = nc.dram_tensor("aggt", [P, B_PER_DEV, H * P], f32,
                          kind="ExternalOutput").ap()

    with tile.TileContext(nc) as tc:
        with (
            tc.tile_pool(name="const", bufs=1) as cpool,
            tc.tile_pool(name="blkin", bufs=3) as bpool,
            tc.tile_pool(name="gath", bufs=2) as gpool,
            tc.tile_pool(name="ework", bufs=3) as epool,
            tc.tile_pool(name="mwork", bufs=4) as mpool,
            tc.tile_pool(name="fin", bufs=2) as fpool,
            tc.tile_pool(name="psum", bufs=2, space="PSUM") as pspool,
        ):
            nc.gpsimd.load_library(mlp)
            iota_sb = cpool.tile([P, P], bf16)
            nc.sync.dma_start(iota_sb[:], iota[:, :])

            for b in range(B_PER_DEV):
                idx_c = bpool.tile([P, t_pb * 8], i16, tag="idx")
                nc.sync.dma_start(idx_c[:], idxw[:, b * t_pb * 8:(b + 1) * t_pb * 8])
                seg_c = bpool.tile([P, t_pb], f32, tag="seg")
                nc.sync.dma_start(seg_c[:], segt[:, b * t_pb:(b + 1) * t_pb])
                ssum_c = bpool.tile([P, 4 * t_pb], f32, tag="ssum")
                nc.sync.dma_start(ssum_c[:], ssum[:, 4 * b * t_pb:4 * (b + 1) * t_pb])

                # ee = exp(-leaky_relu(ssum)); leaky = max(x, 0.2x)
                t0 = epool.tile([P, 4 * t_pb], f32, tag="t0")
                nc.vector.tensor_scalar(out=t0[:], in0=ssum_c[:],
                                        scalar1=0.2, scalar2=None,
                                        op0=mybir.AluOpType.mult)
                t1 = epool.tile([P, 4 * t_pb], f32, tag="t1")
                nc.vector.tensor_tensor(out=t1[:], in0=ssum_c[:], in1=t0[:],
                                        op=mybir.AluOpType.max)
                e_c = epool.tile([P, 4 * t_pb], f32, tag="ec")
                nc.scalar.activation(e_c[:], t1[:],
                                     mybir.ActivationFunctionType.Exp,
                                     bias=0.0, scale=-1.0)

                # gather all of the block's x rows in one dma_gather
                xg_c = gpool.tile([P, t_pb * F], bf16, tag="xg")
                nc.gpsimd.dma_gather(
                    out_ap=xg_c[:].rearrange("p (k f) -> p k f", k=t_pb),
                    in_ap=xtab[:],
                    idxs_ap=idx_c[:],
                    num_idxs=t_pb * P,
                    num_idxs_reg=t_pb * P,
                    elem_size=F,
                    single_packet=False,
                )

                agg_ps = pspool.tile([P, H * P], f32, tag="agg")
                for t in range(t_pb):
                    wall = mpool.tile([P, H * P], bf16, tag="wall")
                    for h in range(H):
                        nc.vector.tensor_scalar(
                            out=wall[:, h * P:(h + 1) * P],
                            in0=iota_sb[:],
                            scalar1=seg_c[:, t:t + 1],
                            scalar2=e_c[:, 4 * t + h:4 * t + h + 1],
                            op0=mybir.AluOpType.is_equal,
                            op1=mybir.AluOpType.mult)
                    nc.tensor.matmul(out=agg_ps[:],
                                     lhsT=xg_c[:, t * F:(t + 1) * F],
                                     rhs=wall[:],
                                     start=(t == 0), stop=(t == t_pb - 1))

                osb = fpool.tile([P, H * P], f32, tag="osb")
                nc.scalar.copy(osb[:], agg_ps[:])
                nc.sync.dma_start(aggt[:, b, :], osb[:])
    nc.compile()
    return nc


def kernel(x, w, a, edge_index):
    global _last_results
    x = np.asarray(x, dtype=np.float32)
    w = np.asarray(w, dtype=np.float32)
    a = np.asarray(a, dtype=np.float32)
    edge_index = np.asarray(edge_index)
    n, f = x.shape

    src = edge_index[0].astype(np.int64)
    dst = edge_index[1].astype(np.int64)

    # host-side tiny projections: s_src/s_dst = x @ (w*a_part) per head
    c_src = (w[:, 0, :] * a[:, :F, 0]).astype(np.float32)
    c_dst = (w[:, 0, :] * a[:, F:, 0]).astype(np.float32)
    s_src = x @ c_src.T  # [N,H]
    s_dst = x @ c_dst.T

    order = np.argsort(src, kind="stable")
    seg = src[order]
    dsts = dst[order]
    ssum_e = (s_src[seg] + s_dst[dsts]).astype(np.float32)  # [E,H]

    # host rowsums (device only produces unnormalized partial aggregates)
    lk = np.where(ssum_e > 0, ssum_e, 0.2 * ssum_e)
    ee = np.exp(-lk)  # [E,H]
    rs = np.zeros((H, NQ * NODES_Q), np.float64)
    for h in range(H):
        rs[h] = np.bincount(seg, weights=ee[:, h], minlength=NQ * NODES_Q)
    rs[rs == 0] = 1.0

    # per-device edge sets: (src quarter, dst half)
    qid = seg // NODES_Q
    hid = (dsts >= HALF).astype(np.int64)
    dev = qid * 2 + hid

    dev_data = []
    max_cnt = 0
    for d in range(NCORES):
        m = dev == d
        seg_d = seg[m]
        dst_d = dsts[m]
        ssum_d = ssum_e[m]
        blk = (seg_d - (d // 2) * NODES_Q) >> 7
        cnt = np.bincount(blk, minlength=B_PER_DEV)
        max_cnt = max(max_cnt, int(cnt.max()))
        dev_data.append((seg_d, dst_d, ssum_d, blk, cnt))

    t_pb = max(1, (max_cnt + P - 1) // P)
    slots_pb = t_pb * P
    T = B_PER_DEV * t_pb

    x_pad = np.zeros((2 * HALF, F), np.float32)
    x_pad[:n] = x
    x_bf = x_pad.astype(ml_dtypes.bfloat16)
    iota_np = np.broadcast_to(
        np.arange(P, dtype=np.float32), (P, P)).astype(ml_dtypes.bfloat16)

    in_maps = []
    for d in range(NCORES):
        seg_d, dst_d, ssum_d, blk, cnt = dev_data[d]
        q, half = d // 2, d % 2
        starts = np.zeros(B_PER_DEV, np.int64)
        np.cumsum(cnt[:-1], out=starts[1:])
        slot = blk * slots_pb + (np.arange(len(seg_d)) - starts[blk])

        nslots = B_PER_DEV * slots_pb
        dst_slots = np.zeros(nslots, np.int16)
        dst_slots[slot] = (dst_d - half * HALF).astype(np.int16)
        seg_slots = np.zeros(nslots, np.float32)
        seg_slots[slot] = (seg_d - q * NODES_Q - (blk << 7)).astype(np.float32)
        ssum_slots = np.full((nslots, H), PAD_SCORE, np.float32)
        ssum_slots[slot] = ssum_d

        # idx wrapped layout: flat i -> [i%16, i//16], replicated to 128 parts
        wq = dst_slots.reshape(B_PER_DEV, -1, 16).transpose(0, 2, 1)
        wq = np.tile(wq, (1, 8, 1))  # [B,128,S/16]
        idxw_np = np.ascontiguousarray(
            wq.transpose(1, 0, 2).reshape(P, -1)).astype(np.int16)

        in_maps.append({
            "xtab": x_bf[half * HALF:(half + 1) * HALF],
            "idxw": idxw_np,
            "segt": np.ascontiguousarray(seg_slots.reshape(T, P).T),
            "ssum": np.ascontiguousarray(
                ssum_slots.reshape(T, P, H).transpose(1, 0, 2).reshape(P, 4 * T)),
            "iota": iota_np,
        })

    if t_pb not in _program_cache:
        _program_cache[t_pb] = _build_program(t_pb)
    nc = _program_cache[t_pb]

    res = run_bass_kernel_spmd(nc, in_maps, core_ids=list(range(NCORES)))
    _last_results = res

    # assemble: aggt [128 f, 98 b, 4 h, 128 i] per device; sum half pairs
    out = np.empty((H, NQ * NODES_Q, F), np.float32)
    for q in range(NQ):
        pair = (res.results[2 * q]["aggt"].astype(np.float64)
                + res.results[2 * q + 1]["aggt"].astype(np.float64))
        ag = pair.reshape(P, B_PER_DEV, H, P).transpose(2, 1, 3, 0)  # h,b,i,f
        out[:, q * NODES_Q:(q + 1) * NODES_Q, :] = ag.reshape(H, NODES_Q, F)
    out *= w[:, 0, :][:, None, :]
    out /= rs[:, :, None]
    return np.ascontiguousarray(out[:, :N_NODES, :]).astype(np.float32)



# revision 6
# speedup vs baseline: 3.7556x; 3.7556x over previous
"""MultiHeadGraphAttention kernel for 8 Trainium2 NeuronCores.

Sharding (2D): 4 src-quarters x 2 dst-halves. Device (q, half) owns edges
with src in quarter q (12544 nodes = 98 blocks of 128) and dst in half
(25024 rows). x is uploaded bf16 as 8 disjoint shards and AllGather'd
on-device into each device's half-table; edges gather x rows via the GPSIMD
dma_gather custom op (int16 indices fit the half-table).

Per 128-edge tile (edges sorted by src within a 128-node block):
  oh[j,i] = (seg_rel[j] == i)                   (one DVE is_equal)
  y[j,(h,f)] = ee[h,j] * xg[j,f]                (broadcast DVE tensor_tensor)
  PSUM_A[i,(h,f)] += oh.T @ y                   (PE matmul, bf16)
  PSUM_R[i,h]     += oh.T @ ee                  (PE matmul, rowsums)
so each device produces PARTIAL per-head aggregates AND rowsums for its
node quarter in [i, h, f] layout (cheap host transpose). The dst-half
pairs are summed on-device with a pair ReduceScatter, so each device
downloads a unique fp16 49-block slice. Host only normalizes:
out = agg * w / rowsum.

Edge scores ssum[e,h] = s_src[src_e,h] + s_dst[dst_e,h] are precomputed on
host (s = x @ (w*a) is a tiny [N,4] projection), shipped as fp16, and the
device computes ee = exp(-leaky_relu(ssum)) in batched DVE/ACT ops.

All per-call jit state is cached module-side: the bass program, the
shard_map-jitted executable, and an on-device zeros generator for the
donated output buffers (avoids re-tracing and avoids uploading zero
buffers over the axon tunnel, which dominated wall time).
"""

import sys

sys.path.insert(0, "/opt/trn_rl_repo")

import ml_dtypes
import numpy as np
import jax
import jax.numpy as jnp
from jax.sharding import Mesh, NamedSharding, PartitionSpec

import concourse.bass as bass  # noqa: F401  (keeps bass registered)
import concourse.tile as tile
from concourse import bacc, bass2jax, mybir
from concourse.library_config import mlp

N_NODES = 50000
H = 4
F = 128
P = 128
NCORES = 8
NQ = 4                      # src quarters
B_PER_DEV = 98              # node blocks per quarter (98*128 = 12544)
NODES_Q = B_PER_DEV * P     # 12544
HALF = 25024                # dst half-table rows (2*25024 = 50048 >= 50000)
XSH = HALF // 4             # x rows uploaded per core (AllGather x4 -> half)
B_LO = B_PER_DEV // 2       # blocks per device after pair ReduceScatter
NGRP = NCORES * B_PER_DEV   # 784 (dev, block) groups
PAD_SCORE = 40.0            # exp(-40) ~ 4e-18: padding edges add nothing

_last_results = None  # test.py introspection
_runner_cache = {}
_mesh = None


def _get_mesh():
    global _mesh
    if _mesh is None:
        _mesh = Mesh(np.asarray(jax.devices()[:NCORES]), ("core",))
    return _mesh


def _build_program(t_pb: int):
    """SPMD program, identical on all 8 cores; t_pb = edge tiles per block."""
    f32 = mybir.dt.float32
    bf16 = mybir.dt.bfloat16
    f16 = mybir.dt.float16
    i16 = mybir.dt.int16
    T = B_PER_DEV * t_pb

    nc = bacc.Bacc("TRN2", target_bir_lowering=False, debug=False,
                   num_devices=NCORES)

    xshard = nc.dram_tensor("xshard", [XSH, F], bf16, kind="ExternalInput").ap()
    idxw16 = nc.dram_tensor("idxw16", [16, T * 8], i16, kind="ExternalInput").ap()
    segt = nc.dram_tensor("segt", [P, T], f32, kind="ExternalInput").ap()
    ssum = nc.dram_tensor("ssum", [P, 4 * T], f16, kind="ExternalInput").ap()
    iota = nc.dram_tensor("iota", [P, P], bf16, kind="ExternalInput").ap()
    xshb = nc.dram_tensor("xshb", [XSH, F], bf16, kind="Internal").ap()
    xtab = nc.dram_tensor("xtab", [HALF, F], bf16, kind="Internal").ap()
    aggf = nc.dram_tensor("aggf", [B_PER_DEV, P, H * P], f16, kind="Internal").ap()
    rsf = nc.dram_tensor("rsf", [B_PER_DEV, P, 4], f16, kind="Internal").ap()
    aggb = nc.dram_tensor("aggb", [B_LO, P, H * P], f16, kind="Internal").ap()
    rsb_d = nc.dram_tensor("rsb_d", [B_LO, P, 4], f16, kind="Internal").ap()
    aggo = nc.dram_tensor("aggo", [B_LO, P, H * P], f16,
                          kind="ExternalOutput").ap()
    rso = nc.dram_tensor("rso", [B_LO, P, 4], f16, kind="ExternalOutput").ap()

    with tile.TileContext(nc) as tc:
        with (
            tc.tile_pool(name="const", bufs=1) as cpool,
            tc.tile_pool(name="gath", bufs=2) as gpool,
            tc.tile_pool(name="ework", bufs=3) as epool,
            tc.tile_pool(name="mwork", bufs=4) as mpool,
            tc.tile_pool(name="fin", bufs=2) as fpool,
            tc.tile_pool(name="psum", bufs=2, space="PSUM") as pspool,
        ):
            nc.gpsimd.load_library(mlp)

            # x AllGather: 4 shards per dst-half -> this device's half table
            # (collectives cannot read IO tensors; bounce through Internal)
            nc.sync.dma_start(xshb[:], xshard[:])
            nc.gpsimd.collective_compute(
                "AllGather", mybir.AluOpType.bypass,
                replica_groups=[[0, 2, 4, 6], [1, 3, 5, 7]],
                ins=[xshb[:]], outs=[xtab[:]],
            )

            iota_sb = cpool.tile([P, P], bf16)
            nc.sync.dma_start(iota_sb[:], iota[:, :])

            # SBUF-resident per-edge metadata, loaded once.
            idx_sb = cpool.tile([P, T * 8], i16)
            nc.sync.dma_start(idx_sb[0:16, :], idxw16[:, :])
            nc.sync.dma_start(idx_sb[16:32, :], idx_sb[0:16, :])
            nc.sync.dma_start(idx_sb[32:64, :], idx_sb[0:32, :])
            nc.sync.dma_start(idx_sb[64:128, :], idx_sb[0:64, :])
            seg_sb = cpool.tile([P, T], f32)
            nc.sync.dma_start(seg_sb[:], segt[:, :])
            ssum_sb = cpool.tile([P, 4 * T], f16)
            nc.sync.dma_start(ssum_sb[:], ssum[:, :])
            # one upfront f16 -> f32 convert for the whole score table
            ssum_f = cpool.tile([P, 4 * T], f32)
            nc.scalar.copy(ssum_f[:], ssum_sb[:])

            for b in range(B_PER_DEV):
                sl4 = slice(4 * t_pb * b, 4 * t_pb * (b + 1))
                # ee = exp(-leaky_relu(ssum)); leaky = max(x, 0.2x)
                t0 = epool.tile([P, 4 * t_pb], f32, tag="t0")
                nc.vector.tensor_scalar(out=t0[:], in0=ssum_f[:, sl4],
                                        scalar1=0.2, scalar2=None,
                                        op0=mybir.AluOpType.mult)
                t1 = epool.tile([P, 4 * t_pb], f32, tag="t1")
                nc.vector.tensor_tensor(out=t1[:], in0=ssum_f[:, sl4],
                                        in1=t0[:], op=mybir.AluOpType.max)
                ee_b = epool.tile([P, 4 * t_pb], bf16, tag="eb")
                nc.scalar.activation(ee_b[:], t1[:],
                                     mybir.ActivationFunctionType.Exp,
                                     bias=0.0, scale=-1.0)

                # gather all of the block's x rows in one dma_gather
                xg = gpool.tile([P, t_pb * F], bf16, tag="xg")
                nc.gpsimd.dma_gather(
                    out_ap=xg[:].rearrange("p (k f) -> p k f", k=t_pb),
                    in_ap=xtab[:],
                    idxs_ap=idx_sb[:, 8 * t_pb * b:8 * t_pb * (b + 1)],
                    num_idxs=t_pb * P,
                    num_idxs_reg=t_pb * P,
                    elem_size=F,
                    single_packet=False,
                )

                agg_ps = pspool.tile([P, H * P], f32, tag="agg")
                rs_ps = pspool.tile([P, 4], f32, tag="rs")
                for t in range(t_pb):
                    oh = mpool.tile([P, P], bf16, tag="oh")
                    nc.vector.tensor_scalar(
                        out=oh[:], in0=iota_sb[:],
                        scalar1=seg_sb[:, b * t_pb + t:b * t_pb + t + 1],
                        scalar2=None, op0=mybir.AluOpType.is_equal)
                    y = mpool.tile([P, H * P], bf16, tag="y")
                    xgt = xg[:, t * F:(t + 1) * F]
                    eet = ee_b[:, 4 * t:4 * t + 4]
                    nc.vector.tensor_tensor(
                        out=y[:].rearrange("p (h f) -> p h f", h=H),
                        in0=xgt.rearrange("p (o f) -> p o f", o=1)
                            .broadcast_to([P, H, F]),
                        in1=eet.rearrange("p (h o) -> p h o", o=1)
                            .broadcast_to([P, H, F]),
                        op=mybir.AluOpType.mult)
                    nc.tensor.matmul(out=agg_ps[:], lhsT=oh[:], rhs=y[:],
                                     start=(t == 0), stop=(t == t_pb - 1))
                    nc.tensor.matmul(out=rs_ps[:], lhsT=oh[:], rhs=eet,
                                     start=(t == 0), stop=(t == t_pb - 1))

                osb = fpool.tile([P, H * P], f16, tag="osb")
                nc.scalar.copy(osb[:], agg_ps[:])
                rsb = fpool.tile([P, 4], f16, tag="rsb")
                nc.scalar.copy(rsb[:], rs_ps[:])
                nc.sync.dma_start(aggf[b], osb[:])
                nc.sync.dma_start(rsf[b], rsb[:])

            # pair-sum the dst halves on device; each device keeps a
            # unique 49-block slice of its quarter's totals
            nc.gpsimd.collective_compute(
                "ReduceScatter", mybir.AluOpType.add,
                replica_groups=[[0, 1], [2, 3], [4, 5], [6, 7]],
                ins=[aggf[:]], outs=[aggb[:]],
            )
            nc.gpsimd.collective_compute(
                "ReduceScatter", mybir.AluOpType.add,
                replica_groups=[[0, 1], [2, 3], [4, 5], [6, 7]],
                ins=[rsf[:]], outs=[rsb_d[:]],
            )
            nc.sync.dma_start(aggo[:], aggb[:])
            nc.sync.dma_start(rso[:], rsb_d[:])
    nc.compile()
    return nc


class _Runner:
    __slots__ = ("nc", "sharded", "zeros", "in_names", "out_names", "n_params")


def _get_runner(t_pb: int) -> _Runner:
    r = _runner_cache.get(t_pb)
    if r is not None:
        return r
    nc = _build_program(t_pb)
    bass2jax.install_neuronx_cc_hook()
    pn = nc.partition_id_tensor.name if nc.partition_id_tensor else None
    in_names, out_names, out_avals = [], [], []
    for alloc in nc.m.functions[0].allocations:
        if not isinstance(alloc, mybir.MemoryLocationSet):
            continue
        name = alloc.memorylocations[0].name
        if alloc.kind == "ExternalInput":
            if name != pn:
                in_names.append(name)
        elif alloc.kind == "ExternalOutput":
            out_names.append(name)
            out_avals.append(jax.core.ShapedArray(
                tuple(alloc.tensor_shape), mybir.dt.np(alloc.dtype)))
    all_names = tuple(in_names + out_names + ([pn] if pn else []))
    n_params = len(in_names)
    n_outs = len(out_names)

    def _body(*args):
        operands = list(args)
        if pn is not None:
            operands.append(bass2jax.partition_id_tensor())
        return tuple(bass2jax._bass_exec_p.bind(
            *operands, out_avals=tuple(out_avals), in_names=all_names,
            out_names=tuple(out_names), lowering_input_output_aliases=(),
            sim_require_finite=True, sim_require_nnan=True, nc=nc))

    from jax.experimental.shard_map import shard_map
    mesh = _get_mesh()
    spec = PartitionSpec("core")
    sharded = jax.jit(
        shard_map(_body, mesh=mesh, in_specs=(spec,) * (n_params + n_outs),
                  out_specs=(spec,) * n_outs, check_rep=False),
        donate_argnums=tuple(range(n_params, n_params + n_outs)),
        keep_unused=True)

    sh = NamedSharding(mesh, spec)
    zshapes = [(NCORES * av.shape[0], *av.shape[1:]) for av in out_avals]
    zdtypes = [av.dtype for av in out_avals]
    zeros = jax.jit(
        lambda: tuple(jnp.zeros(s, d) for s, d in zip(zshapes, zdtypes)),
        out_shardings=(sh,) * n_outs)

    r = _Runner()
    r.nc, r.sharded, r.zeros = nc, sharded, zeros
    r.in_names, r.out_names, r.n_params = in_names, out_names, n_params
    _runner_cache[t_pb] = r
    return r


def kernel(x, w, a, edge_index):
    global _last_results
    _last_results = None
    x = np.asarray(x, dtype=np.float32)
    w = np.asarray(w, dtype=np.float32)
    a = np.asarray(a, dtype=np.float32)
    edge_index = np.asarray(edge_index)
    n = x.shape[0]

    sh = NamedSharding(_get_mesh(), PartitionSpec("core"))

    # ship x early so the upload overlaps the host-side edge preprocessing
    x_pad = np.zeros((2 * HALF, F), np.float32)
    x_pad[:n] = x
    x_bf = x_pad.astype(ml_dtypes.bfloat16)
    xg_np = np.ascontiguousarray(
        x_bf.reshape(2, 4, XSH, F).transpose(1, 0, 2, 3)).reshape(-1, F)
    xg_dev = jax.device_put(xg_np, sh)

    src = edge_index[0].astype(np.int32)
    dst = edge_index[1].astype(np.int32)
    E = src.shape[0]

    half = (dst >= HALF).astype(np.int32)
    grp = ((src // NODES_Q) * 2 + half) * B_PER_DEV + ((src % NODES_Q) >> 7)
    order = np.argsort(grp, kind="stable")
    g_s = grp[order]
    src_s = src[order]
    dst_s = dst[order]

    counts = np.bincount(grp, minlength=NGRP)
    t_pb = max(1, (int(counts.max()) + P - 1) // P)
    spb = t_pb * P
    T = B_PER_DEV * t_pb
    starts = np.zeros(NGRP, np.int64)
    np.cumsum(counts[:-1], out=starts[1:])
    slot = g_s.astype(np.int64) * spb + (np.arange(E, dtype=np.int64)
                                         - starts[g_s])
    nslots = NGRP * spb

    # tiny per-node projections: s = x @ (w*a_part).T per head
    c_src = (w[:, 0, :] * a[:, :F, 0]).astype(np.float32)
    c_dst = (w[:, 0, :] * a[:, F:, 0]).astype(np.float32)
    s_src = x @ c_src.T  # [N,H]
    s_dst = x @ c_dst.T

    half_s = (g_s // B_PER_DEV) & 1
    dst_slots = np.zeros(nslots, np.int16)
    dst_slots[slot] = (dst_s - half_s * HALF).astype(np.int16)
    seg_slots = np.zeros(nslots, np.float32)
    seg_slots[slot] = (src_s & 127).astype(np.float32)
    ssum_slots = np.full((nslots, H), PAD_SCORE, np.float16)
    ssum_slots[slot] = (s_src[src_s] + s_dst[dst_s]).astype(np.float16)

    # device layouts (global, core-major along axis 0)
    idxw_np = np.ascontiguousarray(
        dst_slots.reshape(NCORES, B_PER_DEV, spb // 16, 16)
        .transpose(0, 3, 1, 2)).reshape(NCORES * 16, B_PER_DEV * (spb // 16))
    segt_np = np.ascontiguousarray(
        seg_slots.reshape(NCORES, T, P).transpose(0, 2, 1)).reshape(
        NCORES * P, T)
    ssum_np = np.ascontiguousarray(
        ssum_slots.reshape(NCORES, T, P, H).transpose(0, 2, 1, 3)).reshape(
        NCORES * P, 4 * T)
    iota_np = np.tile(np.broadcast_to(
        np.arange(P, dtype=np.float32), (P, P)).astype(ml_dtypes.bfloat16),
        (NCORES, 1))

    runner = _get_runner(t_pb)
    in_np = {"xshard": xg_dev, "idxw16": idxw_np, "segt": segt_np,
             "ssum": ssum_np, "iota": iota_np}
    ins = [in_np[name] if isinstance(in_np[name], jax.Array)
           else jax.device_put(in_np[name], sh) for name in runner.in_names]
    outs = runner.sharded(*ins, *runner.zeros())
    out_np = {name: np.asarray(o) for name, o in zip(runner.out_names, outs)}

    # assemble: cores 2q,2q+1 hold blocks [0:49) and [49:98) of quarter q
    agg = out_np["aggo"].reshape(NQ, 2 * B_LO, P, H, F)
    rs = out_np["rso"].reshape(NQ, 2 * B_LO, P, H)
    agg_t = agg.transpose(3, 0, 1, 2, 4).reshape(H, NQ * NODES_Q, F) \
        .astype(np.float32)
    rs_t = rs.transpose(3, 0, 1, 2).reshape(H, NQ * NODES_Q) \
        .astype(np.float32)
    rs_t[rs_t == 0] = 1.0
    agg_t *= w[:, 0, :][:, None, :]
    agg_t /= rs_t[:, :, None]
    return np.ascontiguousarray(agg_t[:, :N_NODES, :])


# revision 9
# speedup vs baseline: 5.1096x; 1.3605x over previous
"""MultiHeadGraphAttention kernel for 8 Trainium2 NeuronCores.

Sharding (2D): 4 src-quarters x 2 dst-halves. Device (q, half) owns edges
with src in quarter q (12544 nodes = 98 blocks of 128) and dst in half
(25024 rows). x is uploaded bf16 as 8 disjoint shards and AllGather'd
on-device into each device's half-table; edges gather x rows via the GPSIMD
dma_gather custom op (int16 indices fit the half-table).

Per 128-edge tile (edges sorted by src within a 128-node block):
  oh[j,i] = (seg_rel[j] == i)                   (one DVE is_equal)
  y[j,(h,f)] = ee[h,j] * xg[j,f]                (broadcast DVE tensor_tensor)
  PSUM_A[i,(h,f)] += oh.T @ y                   (PE matmul, bf16)
  PSUM_R[i,h]     += oh.T @ ee                  (PE matmul, rowsums)
so each device produces PARTIAL per-head aggregates [h,b,i,f] AND rowsums
for its node quarter. The dst-half pairs are combined on-device: rowsums
via a pair AllReduce (downloaded, tiny), aggregates via a pair
ReduceScatter that head-splits [4,98,128,128] -> [2,98,128,128], so each
device downloads a unique fp16 slice that maps to the final [h,n,f] layout
with no host transpose. Host only casts + multiplies by w / rowsum, with
per-shard normalization overlapped with the (bandwidth-bound) fetch.

Edge scores ssum[e,h] = s_src[src_e,h] + s_dst[dst_e,h] are precomputed on
host (s = x @ (w*a) is a tiny [N,4] projection; scores are O(0.3) so bf16
is plenty), shipped bf16, and the device computes ee = exp(-leaky_relu(s))
in batched DVE/ACT ops.

All per-call jit state is cached module-side: the bass program, the
shard_map-jitted executable, and an on-device zeros generator for the
donated output buffers (avoids re-tracing and avoids uploading zero
buffers over the axon tunnel, which dominated wall time).
"""

import sys

sys.path.insert(0, "/opt/trn_rl_repo")

import concurrent.futures as _cf

import ml_dtypes
import numpy as np
import jax
import jax.numpy as jnp
from jax.sharding import Mesh, NamedSharding, PartitionSpec

import concourse.bass as bass  # noqa: F401  (keeps bass registered)
import concourse.tile as tile
from concourse import bacc, bass2jax, mybir
from concourse.library_config import mlp

N_NODES = 50000
H = 4
F = 128
P = 128
NCORES = 8
NQ = 4                      # src quarters
B_PER_DEV = 98              # node blocks per quarter (98*128 = 12544)
NODES_Q = B_PER_DEV * P     # 12544
HALF = 25024                # dst half-table rows (2*25024 = 50048 >= 50000)
XSH = HALF // 4             # x rows uploaded per core (AllGather x4 -> half)
NGRP = NCORES * B_PER_DEV   # 784 (dev, block) groups
PAD_SCORE = 40.0            # exp(-40) ~ 4e-18: padding edges add nothing

_last_results = None  # test.py introspection
_runner_cache = {}
_mesh = None


def _get_mesh():
    global _mesh
    if _mesh is None:
        _mesh = Mesh(np.asarray(jax.devices()[:NCORES]), ("core",))
    return _mesh


def _build_program(t_pb: int):
    """SPMD program, identical on all 8 cores; t_pb = edge tiles per block."""
    f32 = mybir.dt.float32
    bf16 = mybir.dt.bfloat16
    f16 = mybir.dt.float16
    i16 = mybir.dt.int16
    T = B_PER_DEV * t_pb

    nc = bacc.Bacc("TRN2", target_bir_lowering=False, debug=False,
                   num_devices=NCORES)

    xshard = nc.dram_tensor("xshard", [XSH, F], bf16, kind="ExternalInput").ap()
    idxw16 = nc.dram_tensor("idxw16", [16, T * 8], i16, kind="ExternalInput").ap()
    segt = nc.dram_tensor("segt", [P, T], bf16, kind="ExternalInput").ap()
    ssum = nc.dram_tensor("ssum", [P, 4 * T], bf16, kind="ExternalInput").ap()
    iota = nc.dram_tensor("iota", [P, P], bf16, kind="ExternalInput").ap()
    xshb = nc.dram_tensor("xshb", [XSH, F], bf16, kind="Internal").ap()
    xtab = nc.dram_tensor("xtab", [HALF, F], bf16, kind="Internal").ap()
    aggf = nc.dram_tensor("aggf", [H, B_PER_DEV, P, F], f16,
                          kind="Internal").ap()
    rsf = nc.dram_tensor("rsf", [B_PER_DEV, P, H], f16, kind="Internal").ap()
    aggb = nc.dram_tensor("aggb", [H // 2, B_PER_DEV, P, F], f16,
                          kind="Internal").ap()
    rst = nc.dram_tensor("rst", [B_PER_DEV, P, H], f16, kind="Internal").ap()
    aggo = nc.dram_tensor("aggo", [H // 2, B_PER_DEV, P, F], f16,
                          kind="ExternalOutput").ap()
    rso = nc.dram_tensor("rso", [B_PER_DEV, P, H], f16,
                         kind="ExternalOutput").ap()

    with tile.TileContext(nc) as tc:
        with (
            tc.tile_pool(name="const", bufs=1) as cpool,
            tc.tile_pool(name="gath", bufs=2) as gpool,
            tc.tile_pool(name="ework", bufs=3) as epool,
            tc.tile_pool(name="mwork", bufs=4) as mpool,
            tc.tile_pool(name="fin", bufs=2) as fpool,
            tc.tile_pool(name="psum", bufs=2, space="PSUM") as pspool,
        ):
            nc.gpsimd.load_library(mlp)

            # x AllGather: 4 shards per dst-half -> this device's half table
            # (collectives cannot touch IO tensors; bounce through Internal)
            nc.sync.dma_start(xshb[:], xshard[:])
            nc.gpsimd.collective_compute(
                "AllGather", mybir.AluOpType.bypass,
                replica_groups=[[0, 2, 4, 6], [1, 3, 5, 7]],
                ins=[xshb[:]], outs=[xtab[:]],
            )

            iota_sb = cpool.tile([P, P], bf16)
            nc.sync.dma_start(iota_sb[:], iota[:, :])

            # SBUF-resident per-edge metadata, loaded once.
            idx_sb = cpool.tile([P, T * 8], i16)
            nc.sync.dma_start(idx_sb[0:16, :], idxw16[:, :])
            nc.sync.dma_start(idx_sb[16:32, :], idx_sb[0:16, :])
            nc.sync.dma_start(idx_sb[32:64, :], idx_sb[0:32, :])
            nc.sync.dma_start(idx_sb[64:128, :], idx_sb[0:64, :])
            seg_sb = cpool.tile([P, T], bf16)
            nc.sync.dma_start(seg_sb[:], segt[:, :])
            seg_f = cpool.tile([P, T], f32)
            nc.scalar.copy(seg_f[:], seg_sb[:])
            ssum_sb = cpool.tile([P, 4 * T], bf16)
            nc.sync.dma_start(ssum_sb[:], ssum[:, :])
            # one upfront bf16 -> f32 convert for the whole score table
            ssum_f = cpool.tile([P, 4 * T], f32)
            nc.scalar.copy(ssum_f[:], ssum_sb[:])

            for b in range(B_PER_DEV):
                sl4 = slice(4 * t_pb * b, 4 * t_pb * (b + 1))
                # ee = exp(-leaky_relu(ssum)); leaky = max(x, 0.2x)
                t0 = epool.tile([P, 4 * t_pb], f32, tag="t0")
                nc.vector.tensor_scalar(out=t0[:], in0=ssum_f[:, sl4],
                                        scalar1=0.2, scalar2=None,
                                        op0=mybir.AluOpType.mult)
                t1 = epool.tile([P, 4 * t_pb], f32, tag="t1")
                nc.vector.tensor_tensor(out=t1[:], in0=ssum_f[:, sl4],
                                        in1=t0[:], op=mybir.AluOpType.max)
                ee_b = epool.tile([P, 4 * t_pb], bf16, tag="eb")
                nc.scalar.activation(ee_b[:], t1[:],
                                     mybir.ActivationFunctionType.Exp,
                                     bias=0.0, scale=-1.0)

                # gather all of the block's x rows in one dma_gather
                xg = gpool.tile([P, t_pb * F], bf16, tag="xg")
                nc.gpsimd.dma_gather(
                    out_ap=xg[:].rearrange("p (k f) -> p k f", k=t_pb),
                    in_ap=xtab[:],
                    idxs_ap=idx_sb[:, 8 * t_pb * b:8 * t_pb * (b + 1)],
                    num_idxs=t_pb * P,
                    num_idxs_reg=t_pb * P,
                    elem_size=F,
                    single_packet=False,
                )

                agg_ps = pspool.tile([P, H * P], f32, tag="agg")
                rs_ps = pspool.tile([P, H], f32, tag="rs")
                for t in range(t_pb):
                    oh = mpool.tile([P, P], bf16, tag="oh")
                    nc.vector.tensor_scalar(
                        out=oh[:], in0=iota_sb[:],
                        scalar1=seg_f[:, b * t_pb + t:b * t_pb + t + 1],
                        scalar2=None, op0=mybir.AluOpType.is_equal)
                    y = mpool.tile([P, H * P], bf16, tag="y")
                    xgt = xg[:, t * F:(t + 1) * F]
                    eet = ee_b[:, 4 * t:4 * t + 4]
                    nc.vector.tensor_tensor(
                        out=y[:].rearrange("p (h f) -> p h f", h=H),
                        in0=xgt.rearrange("p (o f) -> p o f", o=1)
                            .broadcast_to([P, H, F]),
                        in1=eet.rearrange("p (h o) -> p h o", o=1)
                            .broadcast_to([P, H, F]),
                        op=mybir.AluOpType.mult)
                    nc.tensor.matmul(out=agg_ps[:], lhsT=oh[:], rhs=y[:],
                                     start=(t == 0), stop=(t == t_pb - 1))
                    nc.tensor.matmul(out=rs_ps[:], lhsT=oh[:], rhs=eet,
                                     start=(t == 0), stop=(t == t_pb - 1))

                osb = fpool.tile([P, H * P], f16, tag="osb")
                nc.scalar.copy(osb[:], agg_ps[:])
                rsb = fpool.tile([P, H], f16, tag="rsb")
                nc.scalar.copy(rsb[:], rs_ps[:])
                nc.sync.dma_start(
                    aggf[:, b, :, :].rearrange("h p f -> p h f"),
                    osb[:].rearrange("p (h f) -> p h f", h=H))
                nc.sync.dma_start(rsf[b], rsb[:])

            # pair-combine the dst halves on device: aggregates head-split
            # via ReduceScatter ([4,98,128,128] -> [2,98,128,128]), rowsums
            # AllReduce'd (tiny, host divides)
            nc.gpsimd.collective_compute(
                "ReduceScatter", mybir.AluOpType.add,
                replica_groups=[[0, 1], [2, 3], [4, 5], [6, 7]],
                ins=[aggf[:]], outs=[aggb[:]],
            )
            nc.gpsimd.collective_compute(
                "AllReduce", mybir.AluOpType.add,
                replica_groups=[[0, 1], [2, 3], [4, 5], [6, 7]],
                ins=[rsf[:]], outs=[rst[:]],
            )
            nc.sync.dma_start(aggo[:], aggb[:])
            nc.sync.dma_start(rso[:], rst[:])
    nc.compile()
    return nc


class _Runner:
    __slots__ = ("nc", "sharded", "zeros", "in_names", "out_names", "n_params")


def _get_runner(t_pb: int) -> _Runner:
    r = _runner_cache.get(t_pb)
    if r is not None:
        return r
    nc = _build_program(t_pb)
    bass2jax.install_neuronx_cc_hook()
    pn = nc.partition_id_tensor.name if nc.partition_id_tensor else None
    in_names, out_names, out_avals = [], [], []
    for alloc in nc.m.functions[0].allocations:
        if not isinstance(alloc, mybir.MemoryLocationSet):
            continue
        name = alloc.memorylocations[0].name
        if alloc.kind == "ExternalInput":
            if name != pn:
                in_names.append(name)
        elif alloc.kind == "ExternalOutput":
            out_names.append(name)
            out_avals.append(jax.core.ShapedArray(
                tuple(alloc.tensor_shape), mybir.dt.np(alloc.dtype)))
    all_names = tuple(in_names + out_names + ([pn] if pn else []))
    n_params = len(in_names)
    n_outs = len(out_names)

    def _body(*args):
        operands = list(args)
        if pn is not None:
            operands.append(bass2jax.partition_id_tensor())
        return tuple(bass2jax._bass_exec_p.bind(
            *operands, out_avals=tuple(out_avals), in_names=all_names,
            out_names=tuple(out_names), lowering_input_output_aliases=(),
            sim_require_finite=True, sim_require_nnan=True, nc=nc))

    from jax.experimental.shard_map import shard_map
    mesh = _get_mesh()
    spec = PartitionSpec("core")
    sharded = jax.jit(
        shard_map(_body, mesh=mesh, in_specs=(spec,) * (n_params + n_outs),
                  out_specs=(spec,) * n_outs, check_rep=False),
        donate_argnums=tuple(range(n_params, n_params + n_outs)),
        keep_unused=True)

    sh = NamedSharding(mesh, spec)
    zshapes = [(NCORES * av.shape[0], *av.shape[1:]) for av in out_avals]
    zdtypes = [av.dtype for av in out_avals]
    zeros = jax.jit(
        lambda: tuple(jnp.zeros(s, d) for s, d in zip(zshapes, zdtypes)),
        out_shardings=(sh,) * n_outs)

    r = _Runner()
    r.nc, r.sharded, r.zeros = nc, sharded, zeros
    r.in_names, r.out_names, r.n_params = in_names, out_names, n_params
    _runner_cache[t_pb] = r
    return r


def kernel(x, w, a, edge_index):
    global _last_results
    _last_results = None
    x = np.asarray(x, dtype=np.float32)
    w = np.asarray(w, dtype=np.float32)
    a = np.asarray(a, dtype=np.float32)
    edge_index = np.asarray(edge_index)
    n = x.shape[0]

    sh = NamedSharding(_get_mesh(), PartitionSpec("core"))

    # ship x early so the upload overlaps the host-side edge preprocessing
    x_pad = np.zeros((2 * HALF, F), np.float32)
    x_pad[:n] = x
    x_bf = x_pad.astype(ml_dtypes.bfloat16)
    xg_np = np.ascontiguousarray(
        x_bf.reshape(2, 4, XSH, F).transpose(1, 0, 2, 3)).reshape(-1, F)
    xg_dev = jax.device_put(xg_np, sh)

    src = edge_index[0].astype(np.int32)
    dst = edge_index[1].astype(np.int32)
    E = src.shape[0]

    half = (dst >= HALF).astype(np.int32)
    grp = ((src // NODES_Q) * 2 + half) * B_PER_DEV + ((src % NODES_Q) >> 7)
    order = np.argsort(grp.astype(np.uint16), kind="stable")
    g_s = grp[order]
    src_s = src[order]
    dst_s = dst[order]

    counts = np.bincount(grp, minlength=NGRP)
    t_pb = max(1, (int(counts.max()) + P - 1) // P)
    spb = t_pb * P
    T = B_PER_DEV * t_pb
    starts = np.zeros(NGRP, np.int64)
    np.cumsum(counts[:-1], out=starts[1:])
    slot = g_s.astype(np.int64) * spb + (np.arange(E, dtype=np.int64)
                                         - starts[g_s])
    nslots = NGRP * spb

    # tiny per-node projections: s = x @ (w*a_part).T per head
    c_src = (w[:, 0, :] * a[:, :F, 0]).astype(np.float32)
    c_dst = (w[:, 0, :] * a[:, F:, 0]).astype(np.float32)
    s_src = x @ c_src.T  # [N,H]
    s_dst = x @ c_dst.T

    half_s = (g_s // B_PER_DEV) & 1
    dst_slots = np.zeros(nslots, np.int16)
    dst_slots[slot] = (dst_s - half_s * HALF).astype(np.int16)
    seg_slots = np.zeros(nslots, np.float32)
    seg_slots[slot] = (src_s & 127).astype(np.float32)
    ssum_slots = np.full((nslots, H), PAD_SCORE, ml_dtypes.bfloat16)
    ssum_slots[slot] = (s_src[src_s] + s_dst[dst_s]).astype(ml_dtypes.bfloat16)

    # device layouts (global, core-major along axis 0)
    idxw_np = np.ascontiguousarray(
        dst_slots.reshape(NCORES, B_PER_DEV, spb // 16, 16)
        .transpose(0, 3, 1, 2)).reshape(NCORES * 16, B_PER_DEV * (spb // 16))
    segt_np = np.ascontiguousarray(
        seg_slots.reshape(NCORES, T, P).transpose(0, 2, 1)
        .astype(ml_dtypes.bfloat16)).reshape(NCORES * P, T)
    ssum_np = np.ascontiguousarray(
        ssum_slots.reshape(NCORES, T, P, H).transpose(0, 2, 1, 3)).reshape(
        NCORES * P, 4 * T)
    iota_np = np.tile(np.broadcast_to(
        np.arange(P, dtype=np.float32), (P, P)).astype(ml_dtypes.bfloat16),
        (NCORES, 1))

    runner = _get_runner(t_pb)
    in_np = {"xshard": xg_dev, "idxw16": idxw_np, "segt": segt_np,
             "ssum": ssum_np, "iota": iota_np}
    ins = [in_np[name] if isinstance(in_np[name], jax.Array)
           else jax.device_put(in_np[name], sh) for name in runner.in_names]
    outs = runner.sharded(*ins, *runner.zeros())
    out_by_name = dict(zip(runner.out_names, outs))

    # rowsums first (tiny): rs[q] from either pair member (AllReduce'd)
    rs_all = np.asarray(out_by_name["rso"]).reshape(
        NCORES, B_PER_DEV, P, H)  # identical within each pair
    rs_q = rs_all[0::2].astype(np.float32).reshape(NQ, NODES_Q, H)
    rs_q[rs_q == 0] = 1.0
    inv_rs = 1.0 / rs_q  # [q, n_local, h]

    # aggregates: core c=2q+r holds heads (2r, 2r+1) of quarter q in
    # [2, 98, 128, 128] = [h', n_local, f] layout -> no transpose needed.
    out_full = np.empty((H, NQ * NODES_Q, F), np.float32)
    aggo = out_by_name["aggo"]
    wn = w[:, 0, :]  # [H, F]

    def _fetch_and_norm(c):
        shard = np.asarray(aggo.addressable_shards[c].data)  # [2,98,128,128]
        q, rr = divmod(c, 2)
        hs = slice(2 * rr, 2 * rr + 2)
        dst_v = out_full[hs, q * NODES_Q:(q + 1) * NODES_Q, :]
        np.multiply(shard.reshape(2, NODES_Q, F),
                    wn[hs][:, None, :], out=dst_v, casting="unsafe")
        dst_v *= inv_rs[q].T[2 * rr:2 * rr + 2, :, None]

    with _cf.ThreadPoolExecutor(4) as ex:
        list(ex.map(_fetch_and_norm, range(NCORES)))

    return np.ascontiguousarray(out_full[:, :N_NODES, :])


# revision 14
# speedup vs baseline: 6.7893x; 1.3287x over previous
"""MultiHeadGraphAttention kernel for 8 Trainium2 NeuronCores.

Sharding (2D): 4 src-quarters x 2 dst-halves. Device (q, half) owns edges
with src in quarter q (12544 nodes = 98 blocks of 128) and dst in half
(25024 rows). x is uploaded bf16 as 8 disjoint shards and AllGather'd
on-device into each device's half-table; edges gather x rows via the GPSIMD
dma_gather custom op (int16 indices fit the half-table).

Per 128-edge tile (edges sorted by src within a 128-node block):
  oh[j,i] = (seg_rel[j] == i)                   (one DVE is_equal)
  y[j,(h,f)] = ee[h,j] * xg[j,f]                (broadcast DVE tensor_tensor)
  PSUM_A[i,(h,f)] += oh.T @ y                   (PE matmul, bf16)
  PSUM_R[i,h]     += oh.T @ ee                  (PE matmul, rowsums)
so each device produces PARTIAL per-head aggregates [h,b,i,f] AND rowsums
for its node quarter. The dst-half pairs are combined on-device: rowsums
via a pair AllReduce (downloaded, tiny), aggregates via a pair
ReduceScatter that head-splits [4,98,128,128] -> [2,98,128,128], so each
device downloads a unique fp16 slice that maps to the final [h,n,f] layout
with no host transpose. Host only casts + multiplies by w / rowsum, with
per-shard normalization overlapped with the (bandwidth-bound) fetch.

Edge scores ssum[e,h] = s_src[src_e,h] + s_dst[dst_e,h] are precomputed on
host (s = x @ (w*a) is a tiny [N,4] projection; scores are O(0.3) so int8
at scale 1/128 is plenty), shipped int8, and the device computes
ee = exp(-leaky_relu(s)) in batched DVE/ACT ops. Padding slots carry
seg = -1, which makes their one-hot row all-zero (no contribution to
aggregates or rowsums), so their score encoding is irrelevant.

All per-call jit state is cached module-side: the bass program, the
shard_map-jitted executable, and an on-device zeros generator for the
donated output buffers (avoids re-tracing and avoids uploading zero
buffers over the axon tunnel, which dominated wall time).
"""

import sys

sys.path.insert(0, "/opt/trn_rl_repo")

import concurrent.futures as _cf

import ml_dtypes
import numpy as np
import jax
import jax.numpy as jnp
from jax.sharding import Mesh, NamedSharding, PartitionSpec

import concourse.bass as bass  # noqa: F401  (keeps bass registered)
import concourse.tile as tile
from concourse import bacc, bass2jax, mybir
from concourse.library_config import mlp

N_NODES = 50000
H = 4
F = 128
P = 128
NCORES = 8
NQ = 4                      # src quarters
B_PER_DEV = 98              # node blocks per quarter (98*128 = 12544)
NODES_Q = B_PER_DEV * P     # 12544
HALF = 25024                # dst half-table rows (2*25024 = 50048 >= 50000)
XSH = HALF // 4             # x rows uploaded per core (AllGather x4 -> half)
NGRP = NCORES * B_PER_DEV   # 784 (dev, block) groups
SSCALE = 128.0              # int8 score quantization: s_int = round(s*128)

_last_results = None  # test.py introspection
_runner_cache = {}
_mesh = None


def _get_mesh():
    global _mesh
    if _mesh is None:
        _mesh = Mesh(np.asarray(jax.devices()[:NCORES]), ("core",))
    return _mesh


def _build_program(t_pb: int):
    """SPMD program, identical on all 8 cores; t_pb = edge tiles per block."""
    f32 = mybir.dt.float32
    bf16 = mybir.dt.bfloat16
    f16 = mybir.dt.float16
    i16 = mybir.dt.int16
    i8 = mybir.dt.int8
    T = B_PER_DEV * t_pb

    nc = bacc.Bacc("TRN2", target_bir_lowering=False, debug=False,
                   num_devices=NCORES)

    xshard = nc.dram_tensor("xshard", [XSH, F], bf16, kind="ExternalInput").ap()
    idxw16 = nc.dram_tensor("idxw16", [16, T * 8], i16, kind="ExternalInput").ap()
    segt = nc.dram_tensor("segt", [P, T], i8, kind="ExternalInput").ap()
    ssum = nc.dram_tensor("ssum", [P, 4 * T], i8, kind="ExternalInput").ap()
    iota = nc.dram_tensor("iota", [P, P], bf16, kind="ExternalInput").ap()
    xshb = nc.dram_tensor("xshb", [XSH, F], bf16, kind="Internal").ap()
    xtab = nc.dram_tensor("xtab", [HALF, F], bf16, kind="Internal").ap()
    aggf = nc.dram_tensor("aggf", [H, B_PER_DEV, P, F], f16,
                          kind="Internal").ap()
    rsf = nc.dram_tensor("rsf", [B_PER_DEV, P, H], f16, kind="Internal").ap()
    aggb = nc.dram_tensor("aggb", [H // 2, B_PER_DEV, P, F], f16,
                          kind="Internal").ap()
    rst = nc.dram_tensor("rst", [B_PER_DEV, P, H], f16, kind="Internal").ap()
    aggo = nc.dram_tensor("aggo", [H // 2, B_PER_DEV, P, F], f16,
                          kind="ExternalOutput").ap()
    rso = nc.dram_tensor("rso", [B_PER_DEV, P, H], f16,
                         kind="ExternalOutput").ap()

    with tile.TileContext(nc) as tc:
        with (
            tc.tile_pool(name="const", bufs=1) as cpool,
            tc.tile_pool(name="gath", bufs=2) as gpool,
            tc.tile_pool(name="ework", bufs=3) as epool,
            tc.tile_pool(name="mwork", bufs=4) as mpool,
            tc.tile_pool(name="fin", bufs=2) as fpool,
            tc.tile_pool(name="psum", bufs=2, space="PSUM") as pspool,
        ):
            nc.gpsimd.load_library(mlp)

            # x AllGather: 4 shards per dst-half -> this device's half table
            # (collectives cannot touch IO tensors; bounce through Internal)
            nc.sync.dma_start(xshb[:], xshard[:])
            nc.gpsimd.collective_compute(
                "AllGather", mybir.AluOpType.bypass,
                replica_groups=[[0, 2, 4, 6], [1, 3, 5, 7]],
                ins=[xshb[:]], outs=[xtab[:]],
            )

            iota_sb = cpool.tile([P, P], bf16)
            nc.sync.dma_start(iota_sb[:], iota[:, :])

            # SBUF-resident per-edge metadata, loaded once.
            idx_sb = cpool.tile([P, T * 8], i16)
            nc.sync.dma_start(idx_sb[0:16, :], idxw16[:, :])
            nc.sync.dma_start(idx_sb[16:32, :], idx_sb[0:16, :])
            nc.sync.dma_start(idx_sb[32:64, :], idx_sb[0:32, :])
            nc.sync.dma_start(idx_sb[64:128, :], idx_sb[0:64, :])
            seg_sb = cpool.tile([P, T], i8)
            nc.sync.dma_start(seg_sb[:], segt[:, :])
            seg_f = cpool.tile([P, T], f32)
            nc.scalar.copy(seg_f[:], seg_sb[:])
            ssum_sb = cpool.tile([P, 4 * T], i8)
            nc.sync.dma_start(ssum_sb[:], ssum[:, :])
            # one upfront int8 -> f32 dequant for the whole score table
            ssum_f = cpool.tile([P, 4 * T], f32)
            nc.scalar.activation(ssum_f[:], ssum_sb[:],
                                 mybir.ActivationFunctionType.Copy,
                                 bias=0.0, scale=1.0 / SSCALE)

            for b in range(B_PER_DEV):
                sl4 = slice(4 * t_pb * b, 4 * t_pb * (b + 1))
                # ee = exp(-leaky_relu(ssum)); leaky = max(x, 0.2x)
                t0 = epool.tile([P, 4 * t_pb], f32, tag="t0")
                nc.vector.tensor_scalar(out=t0[:], in0=ssum_f[:, sl4],
                                        scalar1=0.2, scalar2=None,
                                        op0=mybir.AluOpType.mult)
                t1 = epool.tile([P, 4 * t_pb], f32, tag="t1")
                nc.vector.tensor_tensor(out=t1[:], in0=ssum_f[:, sl4],
                                        in1=t0[:], op=mybir.AluOpType.max)
                ee_b = epool.tile([P, 4 * t_pb], bf16, tag="eb")
                nc.scalar.activation(ee_b[:], t1[:],
                                     mybir.ActivationFunctionType.Exp,
                                     bias=0.0, scale=-1.0)

                # gather all of the block's x rows in one dma_gather
                xg = gpool.tile([P, t_pb * F], bf16, tag="xg")
                nc.gpsimd.dma_gather(
                    out_ap=xg[:].rearrange("p (k f) -> p k f", k=t_pb),
                    in_ap=xtab[:],
                    idxs_ap=idx_sb[:, 8 * t_pb * b:8 * t_pb * (b + 1)],
                    num_idxs=t_pb * P,
                    num_idxs_reg=t_pb * P,
                    elem_size=F,
                    single_packet=False,
                )

                agg_ps = pspool.tile([P, H * P], f32, tag="agg")
                rs_ps = pspool.tile([P, H], f32, tag="rs")
                for t in range(t_pb):
                    oh = mpool.tile([P, P], bf16, tag="oh")
                    nc.vector.tensor_scalar(
                        out=oh[:], in0=iota_sb[:],
                        scalar1=seg_f[:, b * t_pb + t:b * t_pb + t + 1],
                        scalar2=None, op0=mybir.AluOpType.is_equal)
                    y = mpool.tile([P, H * P], bf16, tag="y")
                    xgt = xg[:, t * F:(t + 1) * F]
                    eet = ee_b[:, 4 * t:4 * t + 4]
                    nc.vector.tensor_tensor(
                        out=y[:].rearrange("p (h f) -> p h f", h=H),
                        in0=xgt.rearrange("p (o f) -> p o f", o=1)
                            .broadcast_to([P, H, F]),
                        in1=eet.rearrange("p (h o) -> p h o", o=1)
                            .broadcast_to([P, H, F]),
                        op=mybir.AluOpType.mult)
                    nc.tensor.matmul(out=agg_ps[:], lhsT=oh[:], rhs=y[:],
                                     start=(t == 0), stop=(t == t_pb - 1))
                    nc.tensor.matmul(out=rs_ps[:], lhsT=oh[:], rhs=eet,
                                     start=(t == 0), stop=(t == t_pb - 1))

                osb = fpool.tile([P, H * P], f16, tag="osb")
                nc.scalar.copy(osb[:], agg_ps[:])
                rsb = fpool.tile([P, H], f16, tag="rsb")
                nc.scalar.copy(rsb[:], rs_ps[:])
                nc.sync.dma_start(
                    aggf[:, b, :, :].rearrange("h p f -> p h f"),
                    osb[:].rearrange("p (h f) -> p h f", h=H))
                nc.sync.dma_start(rsf[b], rsb[:])

            # pair-combine the dst halves on device: aggregates head-split
            # via ReduceScatter ([4,98,128,128] -> [2,98,128,128]), rowsums
            # AllReduce'd (tiny, host divides)
            nc.gpsimd.collective_compute(
                "ReduceScatter", mybir.AluOpType.add,
                replica_groups=[[0, 1], [2, 3], [4, 5], [6, 7]],
                ins=[aggf[:]], outs=[aggb[:]],
            )
            nc.gpsimd.collective_compute(
                "AllReduce", mybir.AluOpType.add,
                replica_groups=[[0, 1], [2, 3], [4, 5], [6, 7]],
                ins=[rsf[:]], outs=[rst[:]],
            )
            nc.sync.dma_start(aggo[:], aggb[:])
            nc.sync.dma_start(rso[:], rst[:])
    nc.compile()
    return nc


class _Runner:
    __slots__ = ("nc", "sharded", "zeros", "in_names", "out_names", "n_params")


def _get_runner(t_pb: int) -> _Runner:
    r = _runner_cache.get(t_pb)
    if r is not None:
        return r
    nc = _build_program(t_pb)
    bass2jax.install_neuronx_cc_hook()
    pn = nc.partition_id_tensor.name if nc.partition_id_tensor else None
    in_names, out_names, out_avals = [], [], []
    for alloc in nc.m.functions[0].allocations:
        if not isinstance(alloc, mybir.MemoryLocationSet):
            continue
        name = alloc.memorylocations[0].name
        if alloc.kind == "ExternalInput":
            if name != pn:
                in_names.append(name)
        elif alloc.kind == "ExternalOutput":
            out_names.append(name)
            out_avals.append(jax.core.ShapedArray(
                tuple(alloc.tensor_shape), mybir.dt.np(alloc.dtype)))
    all_names = tuple(in_names + out_names + ([pn] if pn else []))
    n_params = len(in_names)
    n_outs = len(out_names)

    def _body(*args):
        operands = list(args)
        if pn is not None:
            operands.append(bass2jax.partition_id_tensor())
        return tuple(bass2jax._bass_exec_p.bind(
            *operands, out_avals=tuple(out_avals), in_names=all_names,
            out_names=tuple(out_names), lowering_input_output_aliases=(),
            sim_require_finite=True, sim_require_nnan=True, nc=nc))

    from jax.experimental.shard_map import shard_map
    mesh = _get_mesh()
    spec = PartitionSpec("core")
    sharded = jax.jit(
        shard_map(_body, mesh=mesh, in_specs=(spec,) * (n_params + n_outs),
                  out_specs=(spec,) * n_outs, check_rep=False),
        donate_argnums=tuple(range(n_params, n_params + n_outs)),
        keep_unused=True)

    sh = NamedSharding(mesh, spec)
    zshapes = [(NCORES * av.shape[0], *av.shape[1:]) for av in out_avals]
    zdtypes = [av.dtype for av in out_avals]
    zeros = jax.jit(
        lambda: tuple(jnp.zeros(s, d) for s, d in zip(zshapes, zdtypes)),
        out_shardings=(sh,) * n_outs)

    r = _Runner()
    r.nc, r.sharded, r.zeros = nc, sharded, zeros
    r.in_names, r.out_names, r.n_params = in_names, out_names, n_params
    _runner_cache[t_pb] = r
    return r


def kernel(x, w, a, edge_index):
    global _last_results
    _last_results = None
    x = np.asarray(x, dtype=np.float32)
    w = np.asarray(w, dtype=np.float32)
    a = np.asarray(a, dtype=np.float32)
    edge_index = np.asarray(edge_index)
    n = x.shape[0]

    sh = NamedSharding(_get_mesh(), PartitionSpec("core"))

    # ship x early so the upload overlaps the host-side edge preprocessing
    x_pad = np.zeros((2 * HALF, F), np.float32)
    x_pad[:n] = x
    x_bf = x_pad.astype(ml_dtypes.bfloat16)
    xg_np = np.ascontiguousarray(
        x_bf.reshape(2, 4, XSH, F).transpose(1, 0, 2, 3)).reshape(-1, F)
    xg_dev = jax.device_put(xg_np, sh)

    src = edge_index[0].astype(np.int32)
    dst = edge_index[1].astype(np.int32)
    E = src.shape[0]

    # tiny per-node projections: s = x @ (w*a_part).T per head
    c_src = (w[:, 0, :] * a[:, :F, 0]).astype(np.float32)
    c_dst = (w[:, 0, :] * a[:, F:, 0]).astype(np.float32)
    s_src = x @ c_src.T  # [N,H]
    s_dst = x @ c_dst.T

    half = (dst >= HALF).astype(np.int32)
    grp = ((src // NODES_Q) * 2 + half) * B_PER_DEV + ((src % NODES_Q) >> 7)
    # pre-sort compact encodings (gathered by `order` below)
    dst_rel = (dst - half * HALF).astype(np.int16)
    seg8 = (src & 127).astype(np.int8)
    sq8 = np.clip(np.rint((s_src[src] + s_dst[dst]) * SSCALE),
                  -127, 127).astype(np.int8)  # [E,H]

    order = np.argsort(grp.astype(np.uint16), kind="stable")
    g_s = grp[order]

    counts = np.bincount(grp, minlength=NGRP)
    t_pb = max(1, (int(counts.max()) + P - 1) // P)
    spb = t_pb * P
    T = B_PER_DEV * t_pb
    starts = np.zeros(NGRP, np.int32)
    np.cumsum(counts[:-1], out=starts[1:])
    slot = g_s * spb + (np.arange(E, dtype=np.int32) - starts[g_s])
    nslots = NGRP * spb

    dst_slots = np.zeros(nslots, np.int16)
    dst_slots[slot] = dst_rel[order]
    seg_slots = np.full(nslots, -1, np.int8)  # -1 = padding: all-zero onehot
    seg_slots[slot] = seg8[order]
    ssum_slots = np.zeros((nslots, H), np.int8)
    ssum_slots[slot] = sq8[order]

    # device layouts (global, core-major along axis 0)
    idxw_np = np.ascontiguousarray(
        dst_slots.reshape(NCORES, B_PER_DEV, spb // 16, 16)
        .transpose(0, 3, 1, 2)).reshape(NCORES * 16, B_PER_DEV * (spb // 16))
    segt_np = np.ascontiguousarray(
        seg_slots.reshape(NCORES, T, P).transpose(0, 2, 1)).reshape(
        NCORES * P, T)
    ssum_np = np.ascontiguousarray(
        ssum_slots.reshape(NCORES, T, P, H).transpose(0, 2, 1, 3)).reshape(
        NCORES * P, 4 * T)
    iota_np = np.tile(np.broadcast_to(
        np.arange(P, dtype=np.float32), (P, P)).astype(ml_dtypes.bfloat16),
        (NCORES, 1))

    runner = _get_runner(t_pb)
    zeros = runner.zeros()  # device-side, dispatched before the uploads
    in_np = {"xshard": xg_dev, "idxw16": idxw_np, "segt": segt_np,
             "ssum": ssum_np, "iota": iota_np}
    ins = [in_np[name] if isinstance(in_np[name], jax.Array)
           else jax.device_put(in_np[name], sh) for name in runner.in_names]
    outs = runner.sharded(*ins, *zeros)
    out_by_name = dict(zip(runner.out_names, outs))

    # aggregates: core c=2q+r holds heads (2r, 2r+1) of quarter q in
    # [2, 98, 128, 128] = [h', n_local, f] layout -> no transpose needed.
    out_full = np.empty((H, N_NODES, F), np.float32)
    aggo = out_by_name["aggo"]
    wn = w[:, 0, :]  # [H, F]
    inv_rs_box = [None]

    def _fetch_rs():
        # rowsums (tiny): rs[q] from either pair member (AllReduce'd)
        rs_all = np.asarray(out_by_name["rso"]).reshape(
            NCORES, B_PER_DEV, P, H)
        rs_q = rs_all[0::2].astype(np.float32).reshape(NQ, NODES_Q, H)
        rs_q[rs_q == 0] = 1.0
        inv_rs_box[0] = 1.0 / rs_q  # [q, n_local, h]

    def _fetch_and_norm(c):
        shard = np.asarray(aggo.addressable_shards[c].data)  # [2,98,128,128]
        q, rr = divmod(c, 2)
        hs = slice(2 * rr, 2 * rr + 2)
        lo = q * NODES_Q
        nn = min(NODES_Q, N_NODES - lo)
        if nn <= 0:
            return
        dst_v = out_full[hs, lo:lo + nn, :]
        np.multiply(shard.reshape(2, NODES_Q, F)[:, :nn],
                    wn[hs][:, None, :], out=dst_v, casting="unsafe")
        dst_v *= inv_rs_box[0][q].T[2 * rr:2 * rr + 2, :nn, None]

    with _cf.ThreadPoolExecutor(4) as ex:
        ex.submit(_fetch_rs).result()
        list(ex.map(_fetch_and_norm, range(NCORES)))

    return out_full


# revision 17
# speedup vs baseline: 8.9696x; 1.3212x over previous
"""MultiHeadGraphAttention kernel for 8 Trainium2 NeuronCores.

Sharding (2D): 4 src-quarters x 2 dst-halves. Device (q, half) owns edges
with src in quarter q (12544 nodes = 98 blocks of 128) and dst in half
(25024 rows). x is uploaded bf16 as 8 disjoint shards and AllGather'd
on-device into each device's half-table; edges gather x rows via the GPSIMD
dma_gather custom op (int16 indices fit the half-table).

Per 128-edge tile (edges sorted by src within a 128-node block):
  oh[j,i] = (seg_rel[j] == i)                   (one DVE is_equal)
  y[j,(h,f)] = ee[h,j] * xg[j,f]                (broadcast DVE tensor_tensor)
  PSUM_A[i,(h,f)] += oh.T @ y                   (PE matmul, bf16)
  PSUM_R[i,h]     += oh.T @ ee                  (PE matmul, rowsums)
so each device produces PARTIAL per-head aggregates [h,b,i,f] AND rowsums
for its node quarter. The dst-half pairs are combined on-device: rowsums
via a pair AllReduce (downloaded, tiny), aggregates via a pair
ReduceScatter that head-splits [4,98,128,128] -> [2,98,128,128], so each
device downloads a unique fp16 slice that maps to the final [h,n,f] layout
with no host transpose. Host only casts + multiplies by w / rowsum, with
per-shard normalization overlapped with the (bandwidth-bound) fetch.

Edge scores ssum[e,h] = s_src[src_e,h] + s_dst[dst_e,h] are precomputed on
host (s = x @ (w*a) is a tiny [N,4] projection; scores are O(0.3) so int8
at scale 1/128 is plenty), shipped int8, and the device computes
ee = exp(-leaky_relu(s)) in batched DVE/ACT ops. Padding slots carry
seg = -1, which makes their one-hot row all-zero (no contribution to
aggregates or rowsums), so their score encoding is irrelevant.

All per-call jit state is cached module-side: the bass program, the
shard_map-jitted executable, and an on-device zeros generator for the
donated output buffers (avoids re-tracing and avoids uploading zero
buffers over the axon tunnel, which dominated wall time).
"""

import sys

sys.path.insert(0, "/opt/trn_rl_repo")

import concurrent.futures as _cf

import ml_dtypes
import numpy as np
import jax
import jax.numpy as jnp
from jax.sharding import Mesh, NamedSharding, PartitionSpec

import concourse.bass as bass  # noqa: F401  (keeps bass registered)
import concourse.tile as tile
from concourse import bacc, bass2jax, mybir
from concourse.library_config import mlp

N_NODES = 50000
H = 4
F = 128
P = 128
NCORES = 8
NQ = 4                      # src quarters
B_PER_DEV = 98              # node blocks per quarter (98*128 = 12544)
NODES_Q = B_PER_DEV * P     # 12544
HALF = 25024                # dst half-table rows (2*25024 = 50048 >= 50000)
XSH = HALF // 4             # x rows uploaded per core (AllGather x4 -> half)
NGRP = NCORES * B_PER_DEV   # 784 (dev, block) groups
SSCALE = 128.0              # int8 score quantization: s_int = round(s*128)

_last_results = None  # test.py introspection
_runner_cache = {}
_mesh = None


def _get_mesh():
    global _mesh
    if _mesh is None:
        _mesh = Mesh(np.asarray(jax.devices()[:NCORES]), ("core",))
    return _mesh


def _build_program(t_pb: int):
    """SPMD program, identical on all 8 cores; t_pb = edge tiles per block."""
    f32 = mybir.dt.float32
    bf16 = mybir.dt.bfloat16
    f16 = mybir.dt.float16
    i16 = mybir.dt.int16
    i8 = mybir.dt.int8
    T = B_PER_DEV * t_pb

    nc = bacc.Bacc("TRN2", target_bir_lowering=False, debug=False,
                   num_devices=NCORES)

    xshard = nc.dram_tensor("xshard", [XSH, F], bf16, kind="ExternalInput").ap()
    idxw16 = nc.dram_tensor("idxw16", [16, T * 8], i16, kind="ExternalInput").ap()
    segt = nc.dram_tensor("segt", [P, T], i8, kind="ExternalInput").ap()
    ssum = nc.dram_tensor("ssum", [P, 4 * T], i8, kind="ExternalInput").ap()
    iota = nc.dram_tensor("iota", [P, P], bf16, kind="ExternalInput").ap()
    xshb = nc.dram_tensor("xshb", [XSH, F], bf16, kind="Internal").ap()
    xtab = nc.dram_tensor("xtab", [HALF, F], bf16, kind="Internal").ap()
    aggf = nc.dram_tensor("aggf", [H, B_PER_DEV, P, F], f16,
                          kind="Internal").ap()
    rsf = nc.dram_tensor("rsf", [B_PER_DEV, P, H], f16, kind="Internal").ap()
    aggb = nc.dram_tensor("aggb", [H // 2, B_PER_DEV, P, F], f16,
                          kind="Internal").ap()
    rst = nc.dram_tensor("rst", [B_PER_DEV, P, H], f16, kind="Internal").ap()
    aggq = nc.dram_tensor("aggq", [H // 2, B_PER_DEV, P, F], i8,
                          kind="ExternalOutput").ap()
    sclo = nc.dram_tensor("sclo", [B_PER_DEV, P, H // 2], f16,
                          kind="ExternalOutput").ap()
    rso = nc.dram_tensor("rso", [B_PER_DEV, P, H], f16,
                         kind="ExternalOutput").ap()

    with tile.TileContext(nc) as tc:
        with (
            tc.tile_pool(name="const", bufs=1) as cpool,
            tc.tile_pool(name="gath", bufs=2) as gpool,
            tc.tile_pool(name="ework", bufs=3) as epool,
            tc.tile_pool(name="mwork", bufs=4) as mpool,
            tc.tile_pool(name="fin", bufs=2) as fpool,
            tc.tile_pool(name="psum", bufs=2, space="PSUM") as pspool,
        ):
            nc.gpsimd.load_library(mlp)

            # x AllGather: 4 shards per dst-half -> this device's half table
            # (collectives cannot touch IO tensors; bounce through Internal)
            nc.sync.dma_start(xshb[:], xshard[:])
            nc.gpsimd.collective_compute(
                "AllGather", mybir.AluOpType.bypass,
                replica_groups=[[0, 2, 4, 6], [1, 3, 5, 7]],
                ins=[xshb[:]], outs=[xtab[:]],
            )

            iota_sb = cpool.tile([P, P], bf16)
            nc.sync.dma_start(iota_sb[:], iota[:, :])

            # SBUF-resident per-edge metadata, loaded once.
            idx_sb = cpool.tile([P, T * 8], i16)
            nc.sync.dma_start(idx_sb[0:16, :], idxw16[:, :])
            nc.sync.dma_start(idx_sb[16:32, :], idx_sb[0:16, :])
            nc.sync.dma_start(idx_sb[32:64, :], idx_sb[0:32, :])
            nc.sync.dma_start(idx_sb[64:128, :], idx_sb[0:64, :])
            seg_sb = cpool.tile([P, T], i8)
            nc.sync.dma_start(seg_sb[:], segt[:, :])
            seg_f = cpool.tile([P, T], f32)
            nc.scalar.copy(seg_f[:], seg_sb[:])
            ssum_sb = cpool.tile([P, 4 * T], i8)
            nc.sync.dma_start(ssum_sb[:], ssum[:, :])
            # one upfront int8 -> f32 dequant for the whole score table
            ssum_f = cpool.tile([P, 4 * T], f32)
            nc.scalar.activation(ssum_f[:], ssum_sb[:],
                                 mybir.ActivationFunctionType.Copy,
                                 bias=0.0, scale=1.0 / SSCALE)

            for b in range(B_PER_DEV):
                sl4 = slice(4 * t_pb * b, 4 * t_pb * (b + 1))
                # ee = exp(-leaky_relu(ssum)); leaky = max(x, 0.2x)
                t0 = epool.tile([P, 4 * t_pb], f32, tag="t0")
                nc.vector.tensor_scalar(out=t0[:], in0=ssum_f[:, sl4],
                                        scalar1=0.2, scalar2=None,
                                        op0=mybir.AluOpType.mult)
                t1 = epool.tile([P, 4 * t_pb], f32, tag="t1")
                nc.vector.tensor_tensor(out=t1[:], in0=ssum_f[:, sl4],
                                        in1=t0[:], op=mybir.AluOpType.max)
                ee_b = epool.tile([P, 4 * t_pb], bf16, tag="eb")
                nc.scalar.activation(ee_b[:], t1[:],
                                     mybir.ActivationFunctionType.Exp,
                                     bias=0.0, scale=-1.0)

                # gather all of the block's x rows in one dma_gather
                xg = gpool.tile([P, t_pb * F], bf16, tag="xg")
                nc.gpsimd.dma_gather(
                    out_ap=xg[:].rearrange("p (k f) -> p k f", k=t_pb),
                    in_ap=xtab[:],
                    idxs_ap=idx_sb[:, 8 * t_pb * b:8 * t_pb * (b + 1)],
                    num_idxs=t_pb * P,
                    num_idxs_reg=t_pb * P,
                    elem_size=F,
                    single_packet=False,
                )

                agg_ps = pspool.tile([P, H * P], f32, tag="agg")
                rs_ps = pspool.tile([P, H], f32, tag="rs")
                for t in range(t_pb):
                    oh = mpool.tile([P, P], bf16, tag="oh")
                    nc.vector.tensor_scalar(
                        out=oh[:], in0=iota_sb[:],
                        scalar1=seg_f[:, b * t_pb + t:b * t_pb + t + 1],
                        scalar2=None, op0=mybir.AluOpType.is_equal)
                    y = mpool.tile([P, H * P], bf16, tag="y")
                    xgt = xg[:, t * F:(t + 1) * F]
                    eet = ee_b[:, 4 * t:4 * t + 4]
                    nc.vector.tensor_tensor(
                        out=y[:].rearrange("p (h f) -> p h f", h=H),
                        in0=xgt.rearrange("p (o f) -> p o f", o=1)
                            .broadcast_to([P, H, F]),
                        in1=eet.rearrange("p (h o) -> p h o", o=1)
                            .broadcast_to([P, H, F]),
                        op=mybir.AluOpType.mult)
                    nc.tensor.matmul(out=agg_ps[:], lhsT=oh[:], rhs=y[:],
                                     start=(t == 0), stop=(t == t_pb - 1))
                    nc.tensor.matmul(out=rs_ps[:], lhsT=oh[:], rhs=eet,
                                     start=(t == 0), stop=(t == t_pb - 1))

                osb = fpool.tile([P, H * P], f16, tag="osb")
                nc.scalar.copy(osb[:], agg_ps[:])
                rsb = fpool.tile([P, H], f16, tag="rsb")
                nc.scalar.copy(rsb[:], rs_ps[:])
                nc.sync.dma_start(
                    aggf[:, b, :, :].rearrange("h p f -> p h f"),
                    osb[:].rearrange("p (h f) -> p h f", h=H))
                nc.sync.dma_start(rsf[b], rsb[:])

            # pair-combine the dst halves on device: aggregates head-split
            # via ReduceScatter ([4,98,128,128] -> [2,98,128,128]), rowsums
            # AllReduce'd (tiny, host divides)
            nc.gpsimd.collective_compute(
                "ReduceScatter", mybir.AluOpType.add,
                replica_groups=[[0, 1], [2, 3], [4, 5], [6, 7]],
                ins=[aggf[:]], outs=[aggb[:]],
            )
            nc.gpsimd.collective_compute(
                "AllReduce", mybir.AluOpType.add,
                replica_groups=[[0, 1], [2, 3], [4, 5], [6, 7]],
                ins=[rsf[:]], outs=[rst[:]],
            )
            nc.sync.dma_start(rso[:], rst[:])

            # int8 quantization of the pair-summed aggregates with a
            # per-(node,head) amax scale: halves the (bandwidth-bound)
            # device->host fetch. Host dequantizes via sclo/127.
            H2 = H // 2
            for b in range(B_PER_DEV):
                ab = mpool.tile([P, H2 * F], f16, tag="qab")
                nc.sync.dma_start(
                    ab[:].rearrange("p (h f) -> p h f", h=H2),
                    aggb[:, b, :, :].rearrange("h p f -> p h f"))
                amx = epool.tile([P, H2], f32, tag="qam")
                nc.vector.tensor_reduce(
                    out=amx[:], in_=ab[:].rearrange("p (h f) -> p h f", h=H2),
                    axis=mybir.AxisListType.X, op=mybir.AluOpType.max,
                    apply_absolute_value=True)
                amc = epool.tile([P, H2], f32, tag="qac")
                nc.vector.tensor_scalar(out=amc[:], in0=amx[:],
                                        scalar1=1e-20, scalar2=None,
                                        op0=mybir.AluOpType.max)
                rcp = epool.tile([P, H2], f32, tag="qrc")
                nc.vector.reciprocal(rcp[:], amc[:])
                qm = epool.tile([P, H2], f32, tag="qqm")
                nc.vector.tensor_scalar(out=qm[:], in0=rcp[:],
                                        scalar1=127.0, scalar2=None,
                                        op0=mybir.AluOpType.mult)
                qv = mpool.tile([P, H2 * F], i8, tag="qqv")
                nc.vector.tensor_tensor(
                    out=qv[:].rearrange("p (h f) -> p h f", h=H2),
                    in0=ab[:].rearrange("p (h f) -> p h f", h=H2),
                    in1=qm[:].rearrange("p (h o) -> p h o", o=1)
                        .broadcast_to([P, H2, F]),
                    op=mybir.AluOpType.mult)
                scb = fpool.tile([P, H2], f16, tag="qsc")
                nc.scalar.copy(scb[:], amc[:])
                nc.sync.dma_start(
                    aggq[:, b, :, :].rearrange("h p f -> p h f"),
                    qv[:].rearrange("p (h f) -> p h f", h=H2))
                nc.sync.dma_start(sclo[b], scb[:])
    nc.compile()
    return nc


class _Runner:
    __slots__ = ("nc", "sharded", "zeros", "in_names", "out_names", "n_params")


def _get_runner(t_pb: int) -> _Runner:
    r = _runner_cache.get(t_pb)
    if r is not None:
        return r
    nc = _build_program(t_pb)
    bass2jax.install_neuronx_cc_hook()
    pn = nc.partition_id_tensor.name if nc.partition_id_tensor else None
    in_names, out_names, out_avals = [], [], []
    for alloc in nc.m.functions[0].allocations:
        if not isinstance(alloc, mybir.MemoryLocationSet):
            continue
        name = alloc.memorylocations[0].name
        if alloc.kind == "ExternalInput":
            if name != pn:
                in_names.append(name)
        elif alloc.kind == "ExternalOutput":
            out_names.append(name)
            out_avals.append(jax.core.ShapedArray(
                tuple(alloc.tensor_shape), mybir.dt.np(alloc.dtype)))
    all_names = tuple(in_names + out_names + ([pn] if pn else []))
    n_params = len(in_names)
    n_outs = len(out_names)

    def _body(*args):
        operands = list(args)
        if pn is not None:
            operands.append(bass2jax.partition_id_tensor())
        return tuple(bass2jax._bass_exec_p.bind(
            *operands, out_avals=tuple(out_avals), in_names=all_names,
            out_names=tuple(out_names), lowering_input_output_aliases=(),
            sim_require_finite=True, sim_require_nnan=True, nc=nc))

    from jax.experimental.shard_map import shard_map
    mesh = _get_mesh()
    spec = PartitionSpec("core")
    sharded = jax.jit(
        shard_map(_body, mesh=mesh, in_specs=(spec,) * (n_params + n_outs),
                  out_specs=(spec,) * n_outs, check_rep=False),
        donate_argnums=tuple(range(n_params, n_params + n_outs)),
        keep_unused=True)

    sh = NamedSharding(mesh, spec)
    zshapes = [(NCORES * av.shape[0], *av.shape[1:]) for av in out_avals]
    zdtypes = [av.dtype for av in out_avals]
    zeros = jax.jit(
        lambda: tuple(jnp.zeros(s, d) for s, d in zip(zshapes, zdtypes)),
        out_shardings=(sh,) * n_outs)

    r = _Runner()
    r.nc, r.sharded, r.zeros = nc, sharded, zeros
    r.in_names, r.out_names, r.n_params = in_names, out_names, n_params
    _runner_cache[t_pb] = r
    return r


def kernel(x, w, a, edge_index):
    global _last_results
    _last_results = None
    x = np.asarray(x, dtype=np.float32)
    w = np.asarray(w, dtype=np.float32)
    a = np.asarray(a, dtype=np.float32)
    edge_index = np.asarray(edge_index)
    n = x.shape[0]

    sh = NamedSharding(_get_mesh(), PartitionSpec("core"))

    # ship x early so the upload overlaps the host-side edge preprocessing
    x_pad = np.zeros((2 * HALF, F), np.float32)
    x_pad[:n] = x
    x_bf = x_pad.astype(ml_dtypes.bfloat16)
    xg_np = np.ascontiguousarray(
        x_bf.reshape(2, 4, XSH, F).transpose(1, 0, 2, 3)).reshape(-1, F)
    xg_dev = jax.device_put(xg_np, sh)

    src = edge_index[0].astype(np.int32)
    dst = edge_index[1].astype(np.int32)
    E = src.shape[0]

    # tiny per-node projections: s = x @ (w*a_part).T per head
    c_src = (w[:, 0, :] * a[:, :F, 0]).astype(np.float32)
    c_dst = (w[:, 0, :] * a[:, F:, 0]).astype(np.float32)
    s_src = x @ c_src.T  # [N,H]
    s_dst = x @ c_dst.T

    half = (dst >= HALF).astype(np.int32)
    grp = ((src // NODES_Q) * 2 + half) * B_PER_DEV + ((src % NODES_Q) >> 7)
    # pre-sort compact encodings (gathered by `order` below)
    dst_rel = (dst - half * HALF).astype(np.int16)
    seg8 = (src & 127).astype(np.int8)
    sq8 = np.clip(np.rint((s_src[src] + s_dst[dst]) * SSCALE),
                  -127, 127).astype(np.int8)  # [E,H]

    order = np.argsort(grp.astype(np.uint16), kind="stable")
    g_s = grp[order]

    counts = np.bincount(grp, minlength=NGRP)
    t_pb = max(1, (int(counts.max()) + P - 1) // P)
    spb = t_pb * P
    T = B_PER_DEV * t_pb
    starts = np.zeros(NGRP, np.int32)
    np.cumsum(counts[:-1], out=starts[1:])
    slot = g_s * spb + (np.arange(E, dtype=np.int32) - starts[g_s])
    nslots = NGRP * spb

    dst_slots = np.zeros(nslots, np.int16)
    dst_slots[slot] = dst_rel[order]
    seg_slots = np.full(nslots, -1, np.int8)  # -1 = padding: all-zero onehot
    seg_slots[slot] = seg8[order]
    ssum_slots = np.zeros((nslots, H), np.int8)
    ssum_slots[slot] = sq8[order]

    # device layouts (global, core-major along axis 0)
    idxw_np = np.ascontiguousarray(
        dst_slots.reshape(NCORES, B_PER_DEV, spb // 16, 16)
        .transpose(0, 3, 1, 2)).reshape(NCORES * 16, B_PER_DEV * (spb // 16))
    segt_np = np.ascontiguousarray(
        seg_slots.reshape(NCORES, T, P).transpose(0, 2, 1)).reshape(
        NCORES * P, T)
    ssum_np = np.ascontiguousarray(
        ssum_slots.reshape(NCORES, T, P, H).transpose(0, 2, 1, 3)).reshape(
        NCORES * P, 4 * T)
    iota_np = np.tile(np.broadcast_to(
        np.arange(P, dtype=np.float32), (P, P)).astype(ml_dtypes.bfloat16),
        (NCORES, 1))

    runner = _get_runner(t_pb)
    zeros = runner.zeros()  # device-side, dispatched before the uploads
    in_np = {"xshard": xg_dev, "idxw16": idxw_np, "segt": segt_np,
             "ssum": ssum_np, "iota": iota_np}
    ins = [in_np[name] if isinstance(in_np[name], jax.Array)
           else jax.device_put(in_np[name], sh) for name in runner.in_names]
    outs = runner.sharded(*ins, *zeros)
    out_by_name = dict(zip(runner.out_names, outs))

    # aggregates: core c=2q+r holds heads (2r, 2r+1) of quarter q in
    # [2, 98, 128, 128] = [h', n_local, f] int8 + per-(node,head) amax
    # scales -> no transpose needed; dequant+normalize fused per shard.
    out_full = np.empty((H, N_NODES, F), np.float32)
    aggq = out_by_name["aggq"]
    sclo = out_by_name["sclo"]
    wn = w[:, 0, :]  # [H, F]
    inv_rs_box = [None]

    def _fetch_rs():
        # rowsums (tiny): rs[q] from either pair member (AllReduce'd)
        rs_all = np.asarray(out_by_name["rso"]).reshape(
            NCORES, B_PER_DEV, P, H)
        rs_q = rs_all[0::2].astype(np.float32).reshape(NQ, NODES_Q, H)
        rs_q[rs_q == 0] = 1.0
        inv_rs_box[0] = 1.0 / rs_q  # [q, n_local, h]

    def _fetch_and_norm(c):
        shard = np.asarray(aggq.addressable_shards[c].data)  # [2,98,128,128]
        scl = np.asarray(sclo.addressable_shards[c].data)    # [98,128,2]
        q, rr = divmod(c, 2)
        hs = slice(2 * rr, 2 * rr + 2)
        lo = q * NODES_Q
        nn = min(NODES_Q, N_NODES - lo)
        if nn <= 0:
            return
        # combined per-(h',node) factor: amax/127 / rowsum
        fac = (scl.reshape(NODES_Q, 2).T[:, :nn].astype(np.float32)
               * (1.0 / 127.0)) \
            * inv_rs_box[0][q].T[2 * rr:2 * rr + 2, :nn]
        dst_v = out_full[hs, lo:lo + nn, :]
        np.multiply(shard.reshape(2, NODES_Q, F)[:, :nn],
                    fac[:, :, None], out=dst_v, casting="unsafe")
        dst_v *= wn[hs][:, None, :]

    with _cf.ThreadPoolExecutor(4) as ex:
        ex.submit(_fetch_rs).result()
        list(ex.map(_fetch_and_norm, range(NCORES)))

    return out_full


# revision 21
# speedup vs baseline: 9.9867x; 1.1134x over previous
"""MultiHeadGraphAttention kernel for 8 Trainium2 NeuronCores.

Sharding (2D): 4 src-quarters x 2 dst-halves. Device (q, half) owns edges
with src in quarter q (12544 nodes = 98 blocks of 128) and dst in half
(25024 rows). x is uploaded bf16 as 8 disjoint shards and AllGather'd
on-device into each device's half-table; edges gather x rows via the GPSIMD
dma_gather custom op (int16 indices fit the half-table).

Per 128-edge tile (edges sorted by src within a 128-node block):
  oh[j,i] = (seg_rel[j] == i)                   (one DVE is_equal)
  y[j,(h,f)] = ee[h,j] * xg[j,f]                (broadcast DVE tensor_tensor)
  PSUM_A[i,(h,f)] += oh.T @ y                   (PE matmul, bf16)
  PSUM_R[i,h]     += oh.T @ ee                  (PE matmul, rowsums)
so each device produces PARTIAL per-head aggregates [h,b,i,f] AND rowsums
for its node quarter. The dst-half pairs are combined on-device: rowsums
via a pair AllReduce (downloaded, tiny), aggregates via a pair
ReduceScatter that head-splits [4,98,128,128] -> [2,98,128,128], so each
device downloads a unique fp16 slice that maps to the final [h,n,f] layout
with no host transpose. Host only casts + multiplies by w / rowsum, with
per-shard normalization overlapped with the (bandwidth-bound) fetch.

Edge scores ssum[e,h] = s_src[src_e,h] + s_dst[dst_e,h] are precomputed on
host (s = x @ (w*a) is a tiny [N,4] projection; scores are O(0.3) so int8
at scale 1/128 is plenty), shipped int8, and the device computes
ee = exp(-leaky_relu(s)) in batched DVE/ACT ops. Padding slots carry
seg = -1, which makes their one-hot row all-zero (no contribution to
aggregates or rowsums), so their score encoding is irrelevant.

All per-call jit state is cached module-side: the bass program, the
shard_map-jitted executable, and an on-device zeros generator for the
donated output buffers (avoids re-tracing and avoids uploading zero
buffers over the axon tunnel, which dominated wall time).
"""

import sys

sys.path.insert(0, "/opt/trn_rl_repo")

import concurrent.futures as _cf

import ml_dtypes
import numpy as np
import jax
import jax.numpy as jnp
from jax.sharding import Mesh, NamedSharding, PartitionSpec

import concourse.bass as bass  # noqa: F401  (keeps bass registered)
import concourse.tile as tile
from concourse import bacc, bass2jax, mybir
from concourse.library_config import mlp

N_NODES = 50000
H = 4
F = 128
P = 128
NCORES = 8
NQ = 4                      # src quarters
B_PER_DEV = 98              # node blocks per quarter (98*128 = 12544)
NODES_Q = B_PER_DEV * P     # 12544
HALF = 25024                # dst half-table rows (2*25024 = 50048 >= 50000)
XSH = HALF // 4             # x rows uploaded per core (AllGather x4 -> half)
NGRP = NCORES * B_PER_DEV   # 784 (dev, block) groups
SSCALE = 128.0              # int8 score quantization: s_int = round(s*128)

_last_results = None  # test.py introspection
_runner_cache = {}
_mesh = None


def _get_mesh():
    global _mesh
    if _mesh is None:
        _mesh = Mesh(np.asarray(jax.devices()[:NCORES]), ("core",))
    return _mesh


def _build_program(t_pb: int):
    """SPMD program, identical on all 8 cores; t_pb = edge tiles per block."""
    f32 = mybir.dt.float32
    bf16 = mybir.dt.bfloat16
    f16 = mybir.dt.float16
    i16 = mybir.dt.int16
    i8 = mybir.dt.int8
    T = B_PER_DEV * t_pb

    nc = bacc.Bacc("TRN2", target_bir_lowering=False, debug=False,
                   num_devices=NCORES)

    xshard = nc.dram_tensor("xshard", [XSH, F], bf16, kind="ExternalInput").ap()
    idxw16 = nc.dram_tensor("idxw16", [16, T * 8], i16, kind="ExternalInput").ap()
    segt = nc.dram_tensor("segt", [P, T], i8, kind="ExternalInput").ap()
    ssum = nc.dram_tensor("ssum", [P, 4 * T], i8, kind="ExternalInput").ap()
    iota = nc.dram_tensor("iota", [P, P], bf16, kind="ExternalInput").ap()
    xshb = nc.dram_tensor("xshb", [XSH, F], bf16, kind="Internal").ap()
    xtab = nc.dram_tensor("xtab", [HALF, F], bf16, kind="Internal").ap()
    aggf = nc.dram_tensor("aggf", [H, B_PER_DEV, P, F], f16,
                          kind="Internal").ap()
    rsf = nc.dram_tensor("rsf", [H, B_PER_DEV, P], f16, kind="Internal").ap()
    aggb = nc.dram_tensor("aggb", [H // 2, B_PER_DEV, P, F], f16,
                          kind="Internal").ap()
    rsh = nc.dram_tensor("rsh", [H // 2, B_PER_DEV, P], f16,
                         kind="Internal").ap()
    aggq = nc.dram_tensor("aggq", [H // 2, B_PER_DEV, P, F], i8,
                          kind="ExternalOutput").ap()
    sclo = nc.dram_tensor("sclo", [B_PER_DEV, P, H // 2], f16,
                          kind="ExternalOutput").ap()

    with tile.TileContext(nc) as tc:
        with (
            tc.tile_pool(name="const", bufs=1) as cpool,
            tc.tile_pool(name="gath", bufs=2) as gpool,
            tc.tile_pool(name="ework", bufs=3) as epool,
            tc.tile_pool(name="mwork", bufs=4) as mpool,
            tc.tile_pool(name="fin", bufs=2) as fpool,
            tc.tile_pool(name="psum", bufs=2, space="PSUM") as pspool,
        ):
            nc.gpsimd.load_library(mlp)

            # x AllGather: 4 shards per dst-half -> this device's half table
            # (collectives cannot touch IO tensors; bounce through Internal)
            nc.sync.dma_start(xshb[:], xshard[:])
            nc.gpsimd.collective_compute(
                "AllGather", mybir.AluOpType.bypass,
                replica_groups=[[0, 2, 4, 6], [1, 3, 5, 7]],
                ins=[xshb[:]], outs=[xtab[:]],
            )

            iota_sb = cpool.tile([P, P], bf16)
            nc.sync.dma_start(iota_sb[:], iota[:, :])

            # SBUF-resident per-edge metadata, loaded once.
            idx_sb = cpool.tile([P, T * 8], i16)
            nc.sync.dma_start(idx_sb[0:16, :], idxw16[:, :])
            nc.sync.dma_start(idx_sb[16:32, :], idx_sb[0:16, :])
            nc.sync.dma_start(idx_sb[32:64, :], idx_sb[0:32, :])
            nc.sync.dma_start(idx_sb[64:128, :], idx_sb[0:64, :])
            seg_sb = cpool.tile([P, T], i8)
            nc.sync.dma_start(seg_sb[:], segt[:, :])
            seg_f = cpool.tile([P, T], f32)
            nc.scalar.copy(seg_f[:], seg_sb[:])
            ssum_sb = cpool.tile([P, 4 * T], i8)
            nc.sync.dma_start(ssum_sb[:], ssum[:, :])
            # one upfront int8 -> f32 dequant for the whole score table
            ssum_f = cpool.tile([P, 4 * T], f32)
            nc.scalar.activation(ssum_f[:], ssum_sb[:],
                                 mybir.ActivationFunctionType.Copy,
                                 bias=0.0, scale=1.0 / SSCALE)

            for b in range(B_PER_DEV):
                sl4 = slice(4 * t_pb * b, 4 * t_pb * (b + 1))
                # ee = exp(-leaky_relu(ssum)); leaky = max(x, 0.2x)
                t0 = epool.tile([P, 4 * t_pb], f32, tag="t0")
                nc.vector.tensor_scalar(out=t0[:], in0=ssum_f[:, sl4],
                                        scalar1=0.2, scalar2=None,
                                        op0=mybir.AluOpType.mult)
                t1 = epool.tile([P, 4 * t_pb], f32, tag="t1")
                nc.vector.tensor_tensor(out=t1[:], in0=ssum_f[:, sl4],
                                        in1=t0[:], op=mybir.AluOpType.max)
                ee_b = epool.tile([P, 4 * t_pb], bf16, tag="eb")
                nc.scalar.activation(ee_b[:], t1[:],
                                     mybir.ActivationFunctionType.Exp,
                                     bias=0.0, scale=-1.0)

                # gather all of the block's x rows in one dma_gather
                xg = gpool.tile([P, t_pb * F], bf16, tag="xg")
                nc.gpsimd.dma_gather(
                    out_ap=xg[:].rearrange("p (k f) -> p k f", k=t_pb),
                    in_ap=xtab[:],
                    idxs_ap=idx_sb[:, 8 * t_pb * b:8 * t_pb * (b + 1)],
                    num_idxs=t_pb * P,
                    num_idxs_reg=t_pb * P,
                    elem_size=F,
                    single_packet=False,
                )

                agg_ps = pspool.tile([P, H * P], f32, tag="agg")
                rs_ps = pspool.tile([P, H], f32, tag="rs")
                for t in range(t_pb):
                    oh = mpool.tile([P, P], bf16, tag="oh")
                    nc.vector.tensor_scalar(
                        out=oh[:], in0=iota_sb[:],
                        scalar1=seg_f[:, b * t_pb + t:b * t_pb + t + 1],
                        scalar2=None, op0=mybir.AluOpType.is_equal)
                    y = mpool.tile([P, H * P], bf16, tag="y")
                    xgt = xg[:, t * F:(t + 1) * F]
                    eet = ee_b[:, 4 * t:4 * t + 4]
                    nc.vector.tensor_tensor(
                        out=y[:].rearrange("p (h f) -> p h f", h=H),
                        in0=xgt.rearrange("p (o f) -> p o f", o=1)
                            .broadcast_to([P, H, F]),
                        in1=eet.rearrange("p (h o) -> p h o", o=1)
                            .broadcast_to([P, H, F]),
                        op=mybir.AluOpType.mult)
                    nc.tensor.matmul(out=agg_ps[:], lhsT=oh[:], rhs=y[:],
                                     start=(t == 0), stop=(t == t_pb - 1))
                    nc.tensor.matmul(out=rs_ps[:], lhsT=oh[:], rhs=eet,
                                     start=(t == 0), stop=(t == t_pb - 1))

                osb = fpool.tile([P, H * P], f16, tag="osb")
                nc.scalar.copy(osb[:], agg_ps[:])
                rsb = fpool.tile([P, H], f16, tag="rsb")
                nc.scalar.copy(rsb[:], rs_ps[:])
                nc.sync.dma_start(
                    aggf[:, b, :, :].rearrange("h p f -> p h f"),
                    osb[:].rearrange("p (h f) -> p h f", h=H))
                nc.sync.dma_start(rsf[:, b, :].rearrange("h p -> p h"),
                                  rsb[:])

            # pair-combine the dst halves on device: aggregates head-split
            # via ReduceScatter ([4,98,128,128] -> [2,98,128,128]), rowsums
            # AllReduce'd (tiny, host divides)
            nc.gpsimd.collective_compute(
                "ReduceScatter", mybir.AluOpType.add,
                replica_groups=[[0, 1], [2, 3], [4, 5], [6, 7]],
                ins=[aggf[:]], outs=[aggb[:]],
            )
            nc.gpsimd.collective_compute(
                "ReduceScatter", mybir.AluOpType.add,
                replica_groups=[[0, 1], [2, 3], [4, 5], [6, 7]],
                ins=[rsf[:]], outs=[rsh[:]],
            )

            # int8 quantization of the pair-summed aggregates with a
            # per-(node,head) amax scale: halves the (bandwidth-bound)
            # device->host fetch. The rowsum division is folded into the
            # downloaded scale: sclo = amax/(127*rowsum); host just
            # multiplies by sclo and w.
            H2 = H // 2
            for b in range(B_PER_DEV):
                ab = mpool.tile([P, H2 * F], f16, tag="qab")
                nc.sync.dma_start(
                    ab[:].rearrange("p (h f) -> p h f", h=H2),
                    aggb[:, b, :, :].rearrange("h p f -> p h f"))
                rs2 = fpool.tile([P, H2], f16, tag="qrs")
                nc.sync.dma_start(rs2[:],
                                  rsh[:, b, :].rearrange("h p -> p h"))
                amx = epool.tile([P, H2], f32, tag="qam")
                nc.vector.tensor_reduce(
                    out=amx[:], in_=ab[:].rearrange("p (h f) -> p h f", h=H2),
                    axis=mybir.AxisListType.X, op=mybir.AluOpType.max,
                    apply_absolute_value=True)
                amc = epool.tile([P, H2], f32, tag="qac")
                nc.vector.tensor_scalar(out=amc[:], in0=amx[:],
                                        scalar1=1e-20, scalar2=None,
                                        op0=mybir.AluOpType.max)
                rcp = epool.tile([P, H2], f32, tag="qrc")
                nc.vector.reciprocal(rcp[:], amc[:])
                qm = epool.tile([P, H2], f32, tag="qqm")
                nc.vector.tensor_scalar(out=qm[:], in0=rcp[:],
                                        scalar1=127.0, scalar2=None,
                                        op0=mybir.AluOpType.mult)
                qv = mpool.tile([P, H2 * F], i8, tag="qqv")
                nc.vector.tensor_tensor(
                    out=qv[:].rearrange("p (h f) -> p h f", h=H2),
                    in0=ab[:].rearrange("p (h f) -> p h f", h=H2),
                    in1=qm[:].rearrange("p (h o) -> p h o", o=1)
                        .broadcast_to([P, H2, F]),
                    op=mybir.AluOpType.mult)
                # sclo = amax/(127*rowsum)
                rsc = epool.tile([P, H2], f32, tag="qr2")
                nc.vector.tensor_scalar(out=rsc[:], in0=rs2[:],
                                        scalar1=1e-20, scalar2=None,
                                        op0=mybir.AluOpType.max)
                rrc = epool.tile([P, H2], f32, tag="qr3")
                nc.vector.reciprocal(rrc[:], rsc[:])
                fac = epool.tile([P, H2], f32, tag="qfc")
                nc.vector.tensor_tensor(out=fac[:], in0=amc[:], in1=rrc[:],
                                        op=mybir.AluOpType.mult)
                scb = fpool.tile([P, H2], f16, tag="qsc")
                nc.scalar.activation(scb[:], fac[:],
                                     mybir.ActivationFunctionType.Copy,
                                     bias=0.0, scale=1.0 / 127.0)
                nc.sync.dma_start(
                    aggq[:, b, :, :].rearrange("h p f -> p h f"),
                    qv[:].rearrange("p (h f) -> p h f", h=H2))
                nc.sync.dma_start(sclo[b], scb[:])
    nc.compile()
    return nc


class _Runner:
    __slots__ = ("nc", "sharded", "zeros", "in_names", "out_names", "n_params")


def _get_runner(t_pb: int) -> _Runner:
    r = _runner_cache.get(t_pb)
    if r is not None:
        return r
    nc = _build_program(t_pb)
    bass2jax.install_neuronx_cc_hook()
    pn = nc.partition_id_tensor.name if nc.partition_id_tensor else None
    in_names, out_names, out_avals = [], [], []
    for alloc in nc.m.functions[0].allocations:
        if not isinstance(alloc, mybir.MemoryLocationSet):
            continue
        name = alloc.memorylocations[0].name
        if alloc.kind == "ExternalInput":
            if name != pn:
                in_names.append(name)
        elif alloc.kind == "ExternalOutput":
            out_names.append(name)
            out_avals.append(jax.core.ShapedArray(
                tuple(alloc.tensor_shape), mybir.dt.np(alloc.dtype)))
    all_names = tuple(in_names + out_names + ([pn] if pn else []))
    n_params = len(in_names)
    n_outs = len(out_names)

    def _body(*args):
        operands = list(args)
        if pn is not None:
            operands.append(bass2jax.partition_id_tensor())
        return tuple(bass2jax._bass_exec_p.bind(
            *operands, out_avals=tuple(out_avals), in_names=all_names,
            out_names=tuple(out_names), lowering_input_output_aliases=(),
            sim_require_finite=True, sim_require_nnan=True, nc=nc))

    from jax.experimental.shard_map import shard_map
    mesh = _get_mesh()
    spec = PartitionSpec("core")
    sharded = jax.jit(
        shard_map(_body, mesh=mesh, in_specs=(spec,) * (n_params + n_outs),
                  out_specs=(spec,) * n_outs, check_rep=False),
        donate_argnums=tuple(range(n_params, n_params + n_outs)),
        keep_unused=True)

    sh = NamedSharding(mesh, spec)
    zshapes = [(NCORES * av.shape[0], *av.shape[1:]) for av in out_avals]
    zdtypes = [av.dtype for av in out_avals]
    zeros = jax.jit(
        lambda: tuple(jnp.zeros(s, d) for s, d in zip(zshapes, zdtypes)),
        out_shardings=(sh,) * n_outs)

    r = _Runner()
    r.nc, r.sharded, r.zeros = nc, sharded, zeros
    r.in_names, r.out_names, r.n_params = in_names, out_names, n_params
    _runner_cache[t_pb] = r
    return r


def kernel(x, w, a, edge_index):
    global _last_results
    _last_results = None
    x = np.asarray(x, dtype=np.float32)
    w = np.asarray(w, dtype=np.float32)
    a = np.asarray(a, dtype=np.float32)
    edge_index = np.asarray(edge_index)
    n = x.shape[0]

    sh = NamedSharding(_get_mesh(), PartitionSpec("core"))

    # ship x early so the upload overlaps the host-side edge preprocessing
    x_pad = np.zeros((2 * HALF, F), np.float32)
    x_pad[:n] = x
    x_bf = x_pad.astype(ml_dtypes.bfloat16)
    xg_np = np.ascontiguousarray(
        x_bf.reshape(2, 4, XSH, F).transpose(1, 0, 2, 3)).reshape(-1, F)
    xg_dev = jax.device_put(xg_np, sh)

    src = edge_index[0].astype(np.int32)
    dst = edge_index[1].astype(np.int32)
    E = src.shape[0]

    # tiny per-node projections: s = x @ (w*a_part).T per head
    c_src = (w[:, 0, :] * a[:, :F, 0]).astype(np.float32)
    c_dst = (w[:, 0, :] * a[:, F:, 0]).astype(np.float32)
    s_src = x @ c_src.T  # [N,H]
    s_dst = x @ c_dst.T

    half = (dst >= HALF).astype(np.int32)
    grp = ((src // NODES_Q) * 2 + half) * B_PER_DEV + ((src % NODES_Q) >> 7)
    # pre-sort compact encodings (gathered by `order` below)
    dst_rel = (dst - half * HALF).astype(np.int16)
    seg8 = (src & 127).astype(np.int8)
    sq8 = np.clip(np.rint((s_src[src] + s_dst[dst]) * SSCALE),
                  -127, 127).astype(np.int8)  # [E,H]

    order = np.argsort(grp.astype(np.uint16), kind="stable")
    g_s = grp[order]

    counts = np.bincount(grp, minlength=NGRP)
    t_pb = max(1, (int(counts.max()) + P - 1) // P)
    spb = t_pb * P
    T = B_PER_DEV * t_pb
    starts = np.zeros(NGRP, np.int32)
    np.cumsum(counts[:-1], out=starts[1:])
    slot = g_s * spb + (np.arange(E, dtype=np.int32) - starts[g_s])
    nslots = NGRP * spb

    dst_slots = np.zeros(nslots, np.int16)
    dst_slots[slot] = dst_rel[order]
    seg_slots = np.full(nslots, -1, np.int8)  # -1 = padding: all-zero onehot
    seg_slots[slot] = seg8[order]
    ssum_slots = np.zeros((nslots, H), np.int8)
    ssum_slots[slot] = sq8[order]

    # device layouts (global, core-major along axis 0)
    idxw_np = np.ascontiguousarray(
        dst_slots.reshape(NCORES, B_PER_DEV, spb // 16, 16)
        .transpose(0, 3, 1, 2)).reshape(NCORES * 16, B_PER_DEV * (spb // 16))
    segt_np = np.ascontiguousarray(
        seg_slots.reshape(NCORES, T, P).transpose(0, 2, 1)).reshape(
        NCORES * P, T)
    ssum_np = np.ascontiguousarray(
        ssum_slots.reshape(NCORES, T, P, H).transpose(0, 2, 1, 3)).reshape(
        NCORES * P, 4 * T)
    iota_np = np.tile(np.broadcast_to(
        np.arange(P, dtype=np.float32), (P, P)).astype(ml_dtypes.bfloat16),
        (NCORES, 1))

    runner = _get_runner(t_pb)
    zeros = runner.zeros()  # device-side, dispatched before the uploads
    in_np = {"xshard": xg_dev, "idxw16": idxw_np, "segt": segt_np,
             "ssum": ssum_np, "iota": iota_np}
    ins = [in_np[name] if isinstance(in_np[name], jax.Array)
           else jax.device_put(in_np[name], sh) for name in runner.in_names]
    outs = runner.sharded(*ins, *zeros)
    out_by_name = dict(zip(runner.out_names, outs))

    # aggregates: core c=2q+r holds heads (2r, 2r+1) of quarter q in
    # [2, 98, 128, 128] = [h', n_local, f] int8 + per-(node,head) combined
    # scales (amax/127/rowsum) -> dequant+normalize fused per shard, no
    # transpose, overlapped with the bandwidth-bound fetch.
    out_full = np.empty((H, N_NODES, F), np.float32)
    aggq = out_by_name["aggq"]
    sclo = out_by_name["sclo"]
    wn = w[:, 0, :]  # [H, F]

    def _fetch_and_norm(c):
        shard = np.asarray(aggq.addressable_shards[c].data)  # [2,98,128,128]
        scl = np.asarray(sclo.addressable_shards[c].data)    # [98,128,2]
        q, rr = divmod(c, 2)
        hs = slice(2 * rr, 2 * rr + 2)
        lo = q * NODES_Q
        nn = min(NODES_Q, N_NODES - lo)
        if nn <= 0:
            return
        fac = scl.reshape(NODES_Q, 2).T[:, :nn].astype(np.float32)
        dst_v = out_full[hs, lo:lo + nn, :]
        np.multiply(shard.reshape(2, NODES_Q, F)[:, :nn],
                    fac[:, :, None], out=dst_v, casting="unsafe")
        dst_v *= wn[hs][:, None, :]

    with _cf.ThreadPoolExecutor(4) as ex:
        list(ex.map(_fetch_and_norm, range(NCORES)))

    return out_full
